# revision 14
# baseline (speedup 1.0000x reference)
"""Trainium2 Bass kernel for nn_DiffKS (differentiable Karplus-Strong).

Structure of the computation:
  y[t] = x[t] - sum_{j=0..5} vals[t,j] * y[t - 1 - z_l[t] - j]
with vals / z_l derived from spline-interpolated delay & coefficient
trajectories.  The feedback lag (1 + z_l + j) is always >= ~93 samples, so
128-sample chunks can be computed as dense banded matmuls against a
512-sample window of past output plus a small within-chunk correction.

Parallel structure (v3):
  - 32 time segments of 2048 samples; each of the 8 cores runs 4 segments
    as INDEPENDENT interleaved chunk-chains (4 chains x 16 rounds).
  - prog TOPS: each chain propagates basis+1 right-hand sides (unit
    initial-window columns + one particular column) through its segment's
    chunked recurrence.  The RHS count is PER-CHAIN (the max initial-window
    footprint over that chain's 8 segments, ~170-390) instead of the global
    max lag (~430).  Only the final 4 ring columns (the segment transfer
    operator T) go to DRAM -- no full response operator streaming.
  - combine (host, tiny): chain the 32 transfer operators to get every
    segment's true initial window.
  - prog SOLVE: re-runs the chunked recurrence with the now-known initial
    windows and a single RHS per segment (cheap N=1 matmuls), writing the
    actual output samples.  Reuses the identical packed weight stream.
  - host: within-chunk correction fix-ups + reorder.

Weights are pre-negated on the host so the serial ring update is a plain
PSUM->SBUF copy (split across the ACT/DVE/Pool engines).  DMA descriptors
are issued round-robin from all four sequencers (the v2 kernel saturated
the Sync sequencer with 565ns-per-issue DMA configs).
"""

import os
import numpy as np

import concourse.bacc as bacc
import concourse.tile as tile
import concourse.mybir as mybir
from concourse.bass_utils import run_bass_kernel_spmd


def _ensure_ntff_hook():
    """The agent image's `antenv` stub lacks `axon_hooks`, which
    `run_bass_kernel_spmd(trace=True)` needs under axon for NTFF capture."""
    try:
        from antenv.axon_hooks import get_axon_ntff_profile_hook  # noqa: F401
        return
    except ImportError:
        pass
    import contextlib
    import ctypes
    import sys
    import types

    so_path = "/opt/axon/libaxon_pjrt.so"
    if not os.path.exists(so_path):
        return
    lib = ctypes.CDLL(so_path)
    if not hasattr(lib, "axon_start_nrt_profile"):
        return
    lib.axon_start_nrt_profile.argtypes = [
        ctypes.POINTER(ctypes.c_int64), ctypes.c_size_t]
    lib.axon_start_nrt_profile.restype = ctypes.c_int64
    lib.axon_stop_nrt_profile.argtypes = [ctypes.c_char_p]
    lib.axon_stop_nrt_profile.restype = ctypes.c_int64

    @contextlib.contextmanager
    def _hook(output_dir, device_ids):
        import jax
        jax.devices()
        if device_ids:
            ids = (ctypes.c_int64 * len(device_ids))(*device_ids)
            rc = lib.axon_start_nrt_profile(ids, len(device_ids))
        else:
            rc = lib.axon_start_nrt_profile(None, 0)
        if rc != 0:
            raise RuntimeError(f"axon_start_nrt_profile rc={rc}")
        try:
            yield
        finally:
            n = lib.axon_stop_nrt_profile(str(output_dir).encode())
            if n <= 0:
                print(f"ntff profile: {n} file(s) written to {output_dir}",
                      file=sys.stderr)

    mod = types.ModuleType("antenv.axon_hooks")
    mod._hook = _hook
    mod.get_axon_ntff_profile_hook = lambda: _hook
    mod.set_axon_ntff_profile_hook = lambda h: setattr(mod, "_hook", h)
    import antenv
    antenv.axon_hooks = mod
    sys.modules["antenv.axon_hooks"] = mod


_ensure_ntff_hook()

F32 = mybir.dt.float32
F16 = mybir.dt.float16

N_SAMPLES = 65536
N_FRAMES = 64
L_ORDER = 5
CHUNK = 128
WIN = 512            # window length the chunk matmuls see (4 ring cols)
RING = 8             # ring columns per chain in SBUF
CORR = 64            # within-chunk correction width (needs z_l >= 63)
N_CORES = 8
CH = 4               # independent chains (segments) per core
CPC = 16             # chunks (rounds) per chain
SEGS = N_CORES * CH  # 32 segments of 2048 samples

# filled by kernel() with per-phase profiling results for the test harness
LAST_RESULTS = {}

_NC_CACHE = {}


# device rhs layout: basis columns [0, basis); one zero pad column; the
# particular column at PIDX (4-byte aligned for the fp16 column update);
# one trailing pad so the total width is even.
def _nr_of(basis):
    pidx = basis + 1 + (basis + 1) % 2
    return pidx + 2 - (basis + 1) % 2, pidx


def _seg_of(s, q):
    """Segment index handled by core s, chain q."""
    return s + N_CORES * q


# ----------------------------------------------------------------------------
# host-side preprocessing
# ----------------------------------------------------------------------------

_SPLINE_CACHE = {}


def _spline_matrix(n_in, n_out):
    """Static [n_out, n_in] natural-cubic-spline interpolation matrix."""
    key = (n_in, n_out)
    if key in _SPLINE_CACHE:
        return _SPLINE_CACHE[key]
    t_in = np.linspace(0.0, 1.0, n_in)
    t_out = np.linspace(0.0, 1.0, n_out)
    n = n_in
    h = t_in[1:] - t_in[:-1]
    R = np.zeros((n - 2, n))
    for i in range(n - 2):
        R[i, i] += 6.0 / h[i]
        R[i, i + 1] += -6.0 / h[i] - 6.0 / h[i + 1]
        R[i, i + 2] += 6.0 / h[i + 1]
    A = (
        np.diag(2.0 * (h[:-1] + h[1:]))
        + np.diag(h[1:-1], 1)
        + np.diag(h[1:-1], -1)
    )
    M = np.zeros((n, n))
    M[1:-1] = np.linalg.solve(A, R)
    idx = np.clip(np.searchsorted(t_in, t_out, side="right") - 1, 0, n - 2)
    dt = t_out - t_in[idx]
    S = np.zeros((n_out, n))
    eye = np.eye(n)
    for r in range(n_out):
        i = idx[r]
        b = (eye[i + 1] - eye[i]) / h[i] - h[i] * (2.0 * M[i] + M[i + 1]) / 6.0
        c = M[i] / 2.0
        d = (M[i + 1] - M[i]) / (6.0 * h[i])
        S[r] = eye[i] + b * dt[r] + c * dt[r] ** 2 + d * dt[r] ** 3
    S = S.astype(np.float32)
    _SPLINE_CACHE[key] = S
    return S


def _preprocess(delay, raw, exc, n_samples):
    sig = 1.0 / (1.0 + np.exp(-np.asarray(raw, np.float32)))
    coeff = sig / sig.sum(-1, keepdims=True)
    S = _spline_matrix(N_FRAMES, n_samples)
    delay_interp = S @ np.asarray(delay, np.float32)
    coeff_interp = S @ coeff
    z_l = np.floor(delay_interp).astype(np.int32)
    alfa = (delay_interp - z_l).astype(np.float32)
    b = coeff_interp
    v0 = -(1.0 - alfa) * b[:, 0]
    vmid = -(alfa[:, None] * b[:, : L_ORDER - 1]
             + (1.0 - alfa)[:, None] * b[:, 1:L_ORDER])
    vL = -alfa * b[:, -1]
    vals = np.concatenate([v0[:, None], vmid, vL[:, None]], 1).astype(np.float32)
    x = np.zeros(n_samples, np.float32)
    exc = np.asarray(exc, np.float32)
    x[: exc.shape[0]] = exc
    return vals, z_l, x


def _build_wts(vals, z_l, n_samples):
    """Dense per-chunk matmul weights in lhsT layout.

    wts[c, 128g + p, m] = W[c][m, 128g + p]   (g = 0..3, window blocks)
    wts[c, 512 + p, m]  = L[c][m, p]          (p < 64, correction block)
    """
    n_chunks = n_samples // CHUNK
    t = np.arange(n_samples)
    lag = 1 + z_l[:, None] + np.arange(6)[None, :]
    assert (lag[:, 0] >= CORR).all(), "delay too small for correction width"
    basis = int(lag.max())
    assert basis <= WIN - CORR, "delay too large for window"
    src = t[:, None] - lag
    i_in_chunk = t % CHUNK
    k_win = WIN + i_in_chunk[:, None] - lag
    wts = np.zeros((n_chunks, 5 * CHUNK, CHUNK), np.float32)
    c_of_t = t // CHUNK
    for j in range(6):
        valid = src[:, j] >= 0
        kw = k_win[:, j]
        in_window = valid & (kw < WIN)
        tw = t[in_window]
        wts[c_of_t[tw], kw[tw], i_in_chunk[tw]] += vals[tw, j]
        in_chunk = valid & (kw >= WIN)
        tc = t[in_chunk]
        kc = kw[tc] - WIN
        assert (kc < CORR).all()
        wts[c_of_t[tc], WIN + kc, i_in_chunk[tc]] += vals[tc, j]
    return wts, basis


def _fold_corr(wts_seg):
    """Fold each chunk's within-chunk correction into the weights of its
    in-segment readers so the ring stores *uncorrected* columns."""
    wts_seg = wts_seg.copy()
    n = wts_seg.shape[0]
    blocks = wts_seg.reshape(n, 5, CHUNK, CHUNK)
    corr_active = np.abs(blocks[:, 4]).reshape(n, -1).max(-1) > 0
    for w in range(n):
        if not corr_active[w]:
            continue
        corrT = blocks[w, 4]
        for r in range(w + 1, min(w + 5, n)):
            g = w - r + 4
            blk = blocks[r, g]
            blk[0:CORR] -= corrT[0:CORR, CORR:] @ blk[CORR:]
    return wts_seg


def _seg_basis(seg_wts_neg):
    """Initial-window footprint (in samples before segment start) actually
    read by each segment's first chunks, from the folded weight blocks."""
    out = []
    for w in seg_wts_neg:
        blocks = w.reshape(CPC, 5, CHUNK, CHUNK)
        b = 0
        for r in range(4):
            for g in range(4 - r):
                blk = blocks[r, g]
                nz = np.nonzero(np.abs(blk).max(axis=1) > 0)[0]
                if nz.size:
                    # window coord 128g+p of chunk r = sample
                    # seg_start + 128r + (128g+p) - 512
                    rel = 128 * g + int(nz.min()) - 512 + 128 * r
                    b = max(b, -rel)
        out.append(b)
    return np.array(out, np.int64)


def _basis_ring0(basis_j, nr_q, g0_q):
    """Initial window columns for one segment: basis b is a unit at window
    position (WIN-basis_j)+b; only window cols g0_q..3 are materialized."""
    r0 = np.zeros((CHUNK, 4 - g0_q, nr_q), np.float16)
    for b in range(basis_j):
        p = (WIN - basis_j) + b
        r0[p % CHUNK, p // CHUNK - g0_q, b] = 1.0
    return r0


# ----------------------------------------------------------------------------
# plan construction (shared across cores; SPMD program)
# ----------------------------------------------------------------------------

def _block_act(seg_wts_neg):
    return np.stack([
        np.abs(w.reshape(CPC, 5, -1)).max(-1) > 0 for w in seg_wts_neg
    ])  # [SEGS, CPC, 5]


def _assign_segments(seg_wts_neg, basis_j):
    """Assign the 32 segments to the (core, chain) grid, minimizing the
    exact PE cost  sum_q sum_r union_blocks(q,r) * nr_q  (matmul free-dim
    work).  Returns assign[s, q] = segment id."""
    act = _block_act(seg_wts_neg)[:, :, :4]  # [SEGS, CPC, 4]

    order = np.argsort(basis_j, kind="stable")
    groups = [order[8 * q: 8 * q + 8].tolist() for q in range(CH)]

    def gcost(g):
        nb = act[list(g)].any(0).sum()
        nr = _nr_of(int(basis_j[list(g)].max()))[0]
        return float(nb) * nr

    # simulated annealing on the exact PE-work objective
    import math
    import random
    rng = random.Random(0)
    cost = sum(gcost(g) for g in groups)
    best = ([list(g) for g in groups], cost)
    iters = 120000
    for it in range(iters):
        T = 2000.0 * (1.0 / 2000.0) ** (it / iters)
        a, b = rng.randrange(CH), rng.randrange(CH)
        if a == b:
            continue
        i, j = rng.randrange(N_CORES), rng.randrange(N_CORES)
        ca0, cb0 = gcost(groups[a]), gcost(groups[b])
        groups[a][i], groups[b][j] = groups[b][j], groups[a][i]
        ca, cb = gcost(groups[a]), gcost(groups[b])
        d = (ca + cb) - (ca0 + cb0)
        if d <= 0 or rng.random() < math.exp(-d / T):
            cost += d
            if cost < best[1]:
                best = ([list(g) for g in groups], cost)
        else:
            groups[a][i], groups[b][j] = groups[b][j], groups[a][i]
    groups = best[0]
    # segment 0 carries the excitation; the program injects x only at
    # chain-0 slots, so its group must be chain 0
    q0 = next(q for q in range(CH) if 0 in groups[q])
    groups[0], groups[q0] = groups[q0], groups[0]
    assign = np.zeros((N_CORES, CH), np.int64)
    for q in range(CH):
        for s in range(N_CORES):
            assign[s, q] = groups[q][s]
    return assign


def _make_plans(seg_wts_neg, assign):
    """Per-slot (q, r) union plans across cores.

    Returns plans[q][r] = wblocks tuple, and corr_y[q][r] = bool (host
    fixup positions)."""
    act = _block_act(seg_wts_neg)
    plans = []
    corr_y = []
    for q in range(CH):
        segs = [assign[s, q] for s in range(N_CORES)]
        u = act[segs].any(0)  # [CPC, 5]
        pq = []
        cq = []
        for r in range(CPC):
            wb = tuple(g for g in range(4) if u[r, g])
            if not wb:
                wb = (3,)
            pq.append(wb)
            cq.append(bool(u[r, 4]))
        plans.append(pq)
        corr_y.append(cq)
    return plans, corr_y


def _pack_weights(seg_wts_neg, plans, assign, s):
    """Pack core s's weight blocks, round-major, partition-major fp16
    layout [128, TOT, 128]."""
    cols = []
    for r in range(CPC):
        for q in range(CH):
            wb = plans[q][r]
            blocks = seg_wts_neg[assign[s, q]].reshape(CPC, 5, CHUNK, CHUNK)
            cols.append(blocks[r, list(wb)])  # [nb, 128, 128]
    flat = np.concatenate(cols, 0)            # [TOT, 128k, 128m]
    return np.ascontiguousarray(
        flat.transpose(1, 0, 2)).astype(np.float16)  # [128, TOT, 128]


def _plan_key(plans):
    return tuple(tuple(pq) for pq in plans)


# ----------------------------------------------------------------------------
# prog TOPS: per-segment transfer operators
# ----------------------------------------------------------------------------

def _build_tops_nc(plans, tot_blocks, chain_basis):
    key = ("T3", _plan_key(plans), tot_blocks, tuple(chain_basis))
    if key in _NC_CACHE:
        return _NC_CACHE[key]
    chain_nr = []
    chain_pidx = []
    chain_g0 = []
    for q in range(CH):
        nr, pidx = _nr_of(int(chain_basis[q]))
        chain_nr.append(nr)
        chain_pidx.append(pidx)
        chain_g0.append((WIN - int(chain_basis[q])) // CHUNK)

    nc = bacc.Bacc("TRN2", target_bir_lowering=False, debug=False,
                   num_devices=N_CORES, enable_partition_id=False)
    wts = nc.dram_tensor("wts", [CHUNK, tot_blocks, CHUNK], F16,
                         kind="ExternalInput")
    xin = nc.dram_tensor("xin", [CHUNK, CH * CPC], F32, kind="ExternalInput")
    ring0 = [
        nc.dram_tensor(f"ring0_{q}", [CHUNK, 4 - chain_g0[q], chain_nr[q]],
                       F16, kind="ExternalInput")
        for q in range(CH)
    ]
    tout = [
        nc.dram_tensor(f"tout_{q}", [CHUNK, 4, chain_nr[q]], F16,
                       kind="ExternalOutput")
        for q in range(CH)
    ]

    with tile.TileContext(nc) as tc:
        with (
            tc.tile_pool(name="state", bufs=1) as state,
            tc.tile_pool(name="wpool", bufs=4) as wpool,
            tc.tile_pool(name="psum", bufs=8, space="PSUM") as ppool,
        ):
            ring = [state.tile([CHUNK, RING, chain_nr[q]], F16,
                               name=f"ring{q}")
                    for q in range(CH)]
            xin_sb = state.tile([CHUNK, CH * CPC], F32)
            # initial-window unit columns, split per window col so round-0
            # matmuls wait only on the columns they actually read
            dmae = [nc.scalar, nc.gpsimd, nc.sync]
            nd = len(dmae)
            di = 0
            for q in range(CH):
                for g in range(chain_g0[q], 4):
                    dmae[di % nd].dma_start(
                        ring[q][:, 4 + g, :],
                        ring0[q][:, g - chain_g0[q], :])
                    di += 1
            nc.sync.dma_start(xin_sb[:], xin[:])

            woff = 0
            for r in range(CPC):
                nbr = sum(len(plans[q][r]) for q in range(CH))
                wtile = wpool.tile([CHUNK, nbr, CHUNK], F16, tag="w")
                if r == 0:
                    so = 0
                    for q in range(CH):
                        nb_q = len(plans[q][0])
                        dmae[q % nd].dma_start(
                            wtile[:, so: so + nb_q, :],
                            wts[:, woff + so: woff + so + nb_q, :])
                        so += nb_q
                else:
                    dmae[(r + di) % nd].dma_start(
                        wtile[:, 0:nbr, :], wts[:, woff: woff + nbr, :])
                woff += nbr
                soff = 0
                for q in range(CH):
                    wb = plans[q][r]
                    nr = chain_nr[q]
                    rc = r % RING
                    psum = ppool.tile([CHUNK, nr], F32, tag=f"acc{q}",
                                      bufs=2)
                    for i, g in enumerate(wb):
                        col = (r + 4 + g) % RING
                        nc.tensor.matmul(
                            psum[:],
                            wtile[:, soff + i, :],
                            ring[q][:, col, :],
                            start=(i == 0),
                            stop=(i == len(wb) - 1),
                        )
                    # serial ring update (weights pre-negated: col = psum + x)
                    # split across ACT / DVE (Pool cannot access PSUM)
                    c1 = (int(0.55 * nr) // 2) * 2
                    nc.scalar.copy(ring[q][:, rc, 0:c1], psum[:, 0:c1])
                    nc.vector.tensor_copy(ring[q][:, rc, c1:nr],
                                          psum[:, c1:nr])
                    if q == 0 and r < 4:
                        slot = r * CH + q
                        pidx = chain_pidx[0]
                        nc.vector.tensor_add(
                            ring[0][:, rc, pidx: pidx + 1],
                            ring[0][:, rc, pidx: pidx + 1],
                            xin_sb[:, slot: slot + 1],
                        )
                    soff += len(wb)
                    # stream the transfer operator (last 4 ring cols) out in
                    # column pairs as soon as they are final
                    if r in (CPC - 3, CPC - 1):
                        base = (r - 1) % RING
                        nc.gpsimd.dma_start(
                            tout[q][:, (r - (CPC - 4)) - 1:
                                    (r - (CPC - 4)) + 1, :],
                            ring[q][:, base: base + 2, :],
                        )
    nc.compile()
    _NC_CACHE[key] = nc
    return nc


# ----------------------------------------------------------------------------
# prog SOLVE: final pass with known initial windows, single RHS
# ----------------------------------------------------------------------------

def _build_solve_nc(plans, tot_blocks):
    key = ("S3", _plan_key(plans), tot_blocks)
    if key in _NC_CACHE:
        return _NC_CACHE[key]
    NWC = 2 * (4 + CPC)  # fp16 col pairs: data at even cols (4B aligned)

    nc = bacc.Bacc("TRN2", target_bir_lowering=False, debug=False,
                   num_devices=N_CORES, enable_partition_id=False)
    wts = nc.dram_tensor("wts", [CHUNK, tot_blocks, CHUNK], F16,
                         kind="ExternalInput")
    xin = nc.dram_tensor("xin", [CHUNK, CH * CPC], F32, kind="ExternalInput")
    win0 = nc.dram_tensor("win0", [CHUNK, CH, 4], F16, kind="ExternalInput")
    yout = nc.dram_tensor("yout", [CHUNK, CH, CPC], F16,
                          kind="ExternalOutput")

    with tile.TileContext(nc) as tc:
        with (
            tc.tile_pool(name="state", bufs=1) as state,
            tc.tile_pool(name="wpool", bufs=4) as wpool,
            tc.tile_pool(name="psum", bufs=8, space="PSUM") as ppool,
        ):
            ywin = state.tile([CHUNK, CH, NWC], F16)
            xin_sb = state.tile([CHUNK, CH * CPC], F32)
            for q in range(CH):
                nc.gpsimd.dma_start(ywin[:, q, 0:8:2], win0[:, q, :])
            nc.sync.dma_start(xin_sb[:], xin[:])

            dmae = [nc.scalar, nc.gpsimd, nc.sync]
            nd = len(dmae)
            woff = 0
            for r in range(CPC):
                nbr = sum(len(plans[q][r]) for q in range(CH))
                wtile = wpool.tile([CHUNK, nbr, CHUNK], F16, tag="w")
                if r == 0:
                    so = 0
                    for q in range(CH):
                        nb_q = len(plans[q][0])
                        dmae[q % nd].dma_start(
                            wtile[:, so: so + nb_q, :],
                            wts[:, woff + so: woff + so + nb_q, :])
                        so += nb_q
                else:
                    dmae[r % nd].dma_start(
                        wtile[:, 0:nbr, :], wts[:, woff: woff + nbr, :])
                woff += nbr
                soff = 0
                for r_q in range(CH):
                    q = r_q
                    wb = plans[q][r]
                    psum = ppool.tile([CHUNK, 1], F32, tag=f"acc{q}",
                                      bufs=2)
                    for i, g in enumerate(wb):
                        col = 2 * (r + g)  # = 2*(4 + (r-4+g))
                        nc.tensor.matmul(
                            psum[:],
                            wtile[:, soff + i, :],
                            ywin[:, q, col: col + 1],
                            start=(i == 0),
                            stop=(i == len(wb) - 1),
                        )
                    slot = r * CH + q
                    if q == 0 and r < 4:
                        # only these slots can carry excitation input
                        nc.vector.scalar_tensor_tensor(
                            out=ywin[:, q, 2 * (4 + r): 2 * (4 + r) + 1],
                            in0=psum[:], scalar=1.0,
                            in1=xin_sb[:, slot: slot + 1],
                            op0=mybir.AluOpType.mult,
                            op1=mybir.AluOpType.add,
                        )
                    elif slot % 2 == 0:
                        nc.scalar.copy(
                            ywin[:, q, 2 * (4 + r): 2 * (4 + r) + 1],
                            psum[:])
                    else:
                        nc.vector.tensor_copy(
                            ywin[:, q, 2 * (4 + r): 2 * (4 + r) + 1],
                            psum[:])
                    soff += len(wb)
            for q in range(CH):
                nc.gpsimd.dma_start(yout[:, q, :],
                                    ywin[:, q, 8: 8 + 2 * CPC: 2])
    nc.compile()
    _NC_CACHE[key] = nc
    return nc


# ----------------------------------------------------------------------------
# host orchestration
# ----------------------------------------------------------------------------

def _run(nc, in_maps, tag):
    trace = bool(int(os.environ.get("DIFFKS_TRACE", "0")))
    kw = {}
    tcs = os.environ.get("DIFFKS_TRACE_CORES", "")
    if trace and tcs:
        kw["trace_cores"] = [int(x) for x in tcs.split(",")]
    res = run_bass_kernel_spmd(
        nc, in_maps, core_ids=list(range(len(in_maps))), trace=trace, **kw
    )
    LAST_RESULTS[tag] = res
    return res.results


def kernel(delay_len_frames, raw_coeff_frames, excitation, n_samples):
    n = int(n_samples)
    assert n == N_SAMPLES, f"kernel hardcoded for {N_SAMPLES}, got {n}"
    LAST_RESULTS.clear()

    vals, z_l, x = _preprocess(delay_len_frames, raw_coeff_frames,
                               excitation, n)
    wts, basis = _build_wts(vals, z_l, n)
    n_chunks = n // CHUNK
    assert n_chunks == SEGS * CPC
    xin_cols = np.ascontiguousarray(x.reshape(n_chunks, CHUNK).T)  # [128, nc]

    # fold corrections, then negate everything (update becomes plain copy)
    seg_wts_neg = [-_fold_corr(wts[j * CPC:(j + 1) * CPC])
                   for j in range(SEGS)]
    basis_j = _seg_basis(seg_wts_neg)
    assign = _assign_segments(seg_wts_neg, basis_j)
    inv = {int(assign[s, q]): (s, q)
           for s in range(N_CORES) for q in range(CH)}
    plans, corr_y = _make_plans(seg_wts_neg, assign)
    tot_blocks = sum(len(plans[q][r]) for r in range(CPC) for q in range(CH))
    chain_basis = [int(max(basis_j[assign[s, q]] for s in range(N_CORES)))
                   for q in range(CH)]
    chain_nr = []
    chain_pidx = []
    chain_g0 = []
    for q in range(CH):
        nr, pidx = _nr_of(chain_basis[q])
        chain_nr.append(nr)
        chain_pidx.append(pidx)
        chain_g0.append((WIN - chain_basis[q]) // CHUNK)

    ncT = _build_tops_nc(plans, tot_blocks, chain_basis)
    packed_wts = [_pack_weights(seg_wts_neg, plans, assign, s)
                  for s in range(N_CORES)]
    in_maps = []
    for s in range(N_CORES):
        xin = np.zeros((CHUNK, CH * CPC), np.float32)
        for r in range(CPC):
            for q in range(CH):
                gchunk = int(assign[s, q]) * CPC + r
                xin[:, r * CH + q] = xin_cols[:, gchunk]
        im = {"wts": packed_wts[s], "xin": xin}
        for q in range(CH):
            im[f"ring0_{q}"] = _basis_ring0(
                int(basis_j[assign[s, q]]), chain_nr[q], chain_g0[q])
        in_maps.append(im)
    outsT = _run(ncT, in_maps, "tops")

    # host combine: apply correction fixups to each segment's transfer
    # operator, then chain them (fp32) to get every initial window
    wins = [np.zeros(WIN, np.float32)]
    for j in range(SEGS):
        s, q = inv[j]
        T = outsT[s][f"tout_{q}"].astype(np.float32)  # [128, 4, nr_q]
        blocks = seg_wts_neg[j].reshape(CPC, 5, CHUNK, CHUNK)
        for k in range(4):
            Lc = blocks[CPC - 4 + k, 4][0:CORR]       # negated lhsT [64, 128]
            if np.any(Lc):
                fix = Lc.T @ T[0:CORR, k, :]          # [128, nr]
                T[CORR:, k, :] += fix[CORR:]
        T = T.transpose(1, 0, 2).reshape(WIN, chain_nr[q])
        bj = int(basis_j[j])
        w_next = T[:, :bj] @ wins[j][WIN - bj:] + T[:, chain_pidx[q]]
        wins.append(w_next.astype(np.float32))

    # final pass: known windows, single RHS
    ncS = _build_solve_nc(plans, tot_blocks)
    in_maps = []
    for s in range(N_CORES):
        xin = np.zeros((CHUNK, CH * CPC), np.float32)
        for r in range(CPC):
            for q in range(CH):
                gchunk = int(assign[s, q]) * CPC + r
                xin[:, r * CH + q] = xin_cols[:, gchunk]
        w0 = np.zeros((CHUNK, CH, 4), np.float16)
        for q in range(CH):
            j = int(assign[s, q])
            w0[:, q, :] = wins[j].reshape(4, CHUNK).T.astype(np.float16)
        in_maps.append({"wts": packed_wts[s], "xin": xin, "win0": w0})
    outsS = _run(ncS, in_maps, "solve")

    y = np.zeros(n, np.float32)
    for s in range(N_CORES):
        yo = np.array(outsS[s]["yout"]).astype(np.float32)  # [128, CH, CPC]
        for q in range(CH):
            j = int(assign[s, q])
            blocks = seg_wts_neg[j].reshape(CPC, 5, CHUNK, CHUNK)
            for r in range(CPC):
                col = yo[:, q, r]
                Lc = blocks[r, 4][0:CORR]           # negated lhsT [64, 128]
                if np.any(Lc):
                    fix = Lc.T @ col[0:CORR]
                    col = col.copy()
                    col[CORR:] += fix[CORR:]
                gchunk = j * CPC + r
                y[gchunk * CHUNK:(gchunk + 1) * CHUNK] = col
    return y.astype(np.float32)


# revision 18
# speedup vs baseline: 1.0714x; 1.0714x over previous
"""Trainium2 Bass kernel for nn_DiffKS (differentiable Karplus-Strong).

Structure of the computation:
  y[t] = x[t] - sum_{j=0..5} vals[t,j] * y[t - 1 - z_l[t] - j]
with vals / z_l derived from spline-interpolated delay & coefficient
trajectories.  The feedback lag (1 + z_l + j) is always >= ~93 samples, so
128-sample chunks can be computed as dense banded matmuls against a
512-sample window of past output plus a small within-chunk correction.

Parallel structure (v3):
  - 32 time segments of 2048 samples; each of the 8 cores runs 4 segments
    as INDEPENDENT interleaved chunk-chains (4 chains x 16 rounds).
  - prog TOPS: each chain propagates basis+1 right-hand sides (unit
    initial-window columns + one particular column) through its segment's
    chunked recurrence.  The RHS count is PER-CHAIN (the max initial-window
    footprint over that chain's 8 segments, ~170-390) instead of the global
    max lag (~430).  Only the final 4 ring columns (the segment transfer
    operator T) go to DRAM -- no full response operator streaming.
  - combine (host, tiny): chain the 32 transfer operators to get every
    segment's true initial window.
  - prog SOLVE: re-runs the chunked recurrence with the now-known initial
    windows and a single RHS per segment (cheap N=1 matmuls), writing the
    actual output samples.  Reuses the identical packed weight stream.
  - host: within-chunk correction fix-ups + reorder.

Weights are pre-negated on the host so the serial ring update is a plain
PSUM->SBUF copy (split across the ACT/DVE/Pool engines).  DMA descriptors
are issued round-robin from all four sequencers (the v2 kernel saturated
the Sync sequencer with 565ns-per-issue DMA configs).
"""

import os
import numpy as np

import concourse.bacc as bacc
import concourse.tile as tile
import concourse.mybir as mybir
from concourse.bass_utils import run_bass_kernel_spmd


def _ensure_ntff_hook():
    """The agent image's `antenv` stub lacks `axon_hooks`, which
    `run_bass_kernel_spmd(trace=True)` needs under axon for NTFF capture."""
    try:
        from antenv.axon_hooks import get_axon_ntff_profile_hook  # noqa: F401
        return
    except ImportError:
        pass
    import contextlib
    import ctypes
    import sys
    import types

    so_path = "/opt/axon/libaxon_pjrt.so"
    if not os.path.exists(so_path):
        return
    lib = ctypes.CDLL(so_path)
    if not hasattr(lib, "axon_start_nrt_profile"):
        return
    lib.axon_start_nrt_profile.argtypes = [
        ctypes.POINTER(ctypes.c_int64), ctypes.c_size_t]
    lib.axon_start_nrt_profile.restype = ctypes.c_int64
    lib.axon_stop_nrt_profile.argtypes = [ctypes.c_char_p]
    lib.axon_stop_nrt_profile.restype = ctypes.c_int64

    @contextlib.contextmanager
    def _hook(output_dir, device_ids):
        import jax
        jax.devices()
        if device_ids:
            ids = (ctypes.c_int64 * len(device_ids))(*device_ids)
            rc = lib.axon_start_nrt_profile(ids, len(device_ids))
        else:
            rc = lib.axon_start_nrt_profile(None, 0)
        if rc != 0:
            raise RuntimeError(f"axon_start_nrt_profile rc={rc}")
        try:
            yield
        finally:
            n = lib.axon_stop_nrt_profile(str(output_dir).encode())
            if n <= 0:
                print(f"ntff profile: {n} file(s) written to {output_dir}",
                      file=sys.stderr)

    mod = types.ModuleType("antenv.axon_hooks")
    mod._hook = _hook
    mod.get_axon_ntff_profile_hook = lambda: _hook
    mod.set_axon_ntff_profile_hook = lambda h: setattr(mod, "_hook", h)
    import antenv
    antenv.axon_hooks = mod
    sys.modules["antenv.axon_hooks"] = mod


_ensure_ntff_hook()

F32 = mybir.dt.float32
F16 = mybir.dt.float16

N_SAMPLES = 65536
N_FRAMES = 64
L_ORDER = 5
CHUNK = 128
WIN = 512            # window length the chunk matmuls see (4 ring cols)
RING = 8             # ring columns per chain in SBUF
CORR = 64            # within-chunk correction width (needs z_l >= 63)
N_CORES = 8
CH = 4               # independent chains (segments) per core
CPC = 16             # chunks (rounds) per chain
SEGS = N_CORES * CH  # 32 segments of 2048 samples

# filled by kernel() with per-phase profiling results for the test harness
LAST_RESULTS = {}

_NC_CACHE = {}


# device rhs layout: basis columns [0, basis); one zero pad column; the
# particular column at PIDX (4-byte aligned for the fp16 column update);
# one trailing pad so the total width is even.
def _nr_of(basis):
    pidx = basis + 1 + (basis + 1) % 2
    return pidx + 2 - (basis + 1) % 2, pidx


def _seg_of(s, q):
    """Segment index handled by core s, chain q."""
    return s + N_CORES * q


# ----------------------------------------------------------------------------
# host-side preprocessing
# ----------------------------------------------------------------------------

_SPLINE_CACHE = {}


def _spline_matrix(n_in, n_out):
    """Static [n_out, n_in] natural-cubic-spline interpolation matrix."""
    key = (n_in, n_out)
    if key in _SPLINE_CACHE:
        return _SPLINE_CACHE[key]
    t_in = np.linspace(0.0, 1.0, n_in)
    t_out = np.linspace(0.0, 1.0, n_out)
    n = n_in
    h = t_in[1:] - t_in[:-1]
    R = np.zeros((n - 2, n))
    for i in range(n - 2):
        R[i, i] += 6.0 / h[i]
        R[i, i + 1] += -6.0 / h[i] - 6.0 / h[i + 1]
        R[i, i + 2] += 6.0 / h[i + 1]
    A = (
        np.diag(2.0 * (h[:-1] + h[1:]))
        + np.diag(h[1:-1], 1)
        + np.diag(h[1:-1], -1)
    )
    M = np.zeros((n, n))
    M[1:-1] = np.linalg.solve(A, R)
    idx = np.clip(np.searchsorted(t_in, t_out, side="right") - 1, 0, n - 2)
    dt = t_out - t_in[idx]
    S = np.zeros((n_out, n))
    eye = np.eye(n)
    for r in range(n_out):
        i = idx[r]
        b = (eye[i + 1] - eye[i]) / h[i] - h[i] * (2.0 * M[i] + M[i + 1]) / 6.0
        c = M[i] / 2.0
        d = (M[i + 1] - M[i]) / (6.0 * h[i])
        S[r] = eye[i] + b * dt[r] + c * dt[r] ** 2 + d * dt[r] ** 3
    S = S.astype(np.float32)
    _SPLINE_CACHE[key] = S
    return S


def _preprocess(delay, raw, exc, n_samples):
    sig = 1.0 / (1.0 + np.exp(-np.asarray(raw, np.float32)))
    coeff = sig / sig.sum(-1, keepdims=True)
    S = _spline_matrix(N_FRAMES, n_samples)
    delay_interp = S @ np.asarray(delay, np.float32)
    coeff_interp = S @ coeff
    z_l = np.floor(delay_interp).astype(np.int32)
    alfa = (delay_interp - z_l).astype(np.float32)
    b = coeff_interp
    v0 = -(1.0 - alfa) * b[:, 0]
    vmid = -(alfa[:, None] * b[:, : L_ORDER - 1]
             + (1.0 - alfa)[:, None] * b[:, 1:L_ORDER])
    vL = -alfa * b[:, -1]
    vals = np.concatenate([v0[:, None], vmid, vL[:, None]], 1).astype(np.float32)
    x = np.zeros(n_samples, np.float32)
    exc = np.asarray(exc, np.float32)
    x[: exc.shape[0]] = exc
    return vals, z_l, x


def _build_wts(vals, z_l, n_samples):
    """Dense per-chunk matmul weights in lhsT layout.

    wts[c, 128g + p, m] = W[c][m, 128g + p]   (g = 0..3, window blocks)
    wts[c, 512 + p, m]  = L[c][m, p]          (p < 64, correction block)
    """
    n_chunks = n_samples // CHUNK
    t = np.arange(n_samples)
    lag = 1 + z_l[:, None] + np.arange(6)[None, :]
    assert (lag[:, 0] >= CORR).all(), "delay too small for correction width"
    basis = int(lag.max())
    assert basis <= WIN - CORR, "delay too large for window"
    src = t[:, None] - lag
    i_in_chunk = t % CHUNK
    k_win = WIN + i_in_chunk[:, None] - lag
    wts = np.zeros((n_chunks, 5 * CHUNK, CHUNK), np.float32)
    c_of_t = t // CHUNK
    for j in range(6):
        valid = src[:, j] >= 0
        kw = k_win[:, j]
        in_window = valid & (kw < WIN)
        tw = t[in_window]
        wts[c_of_t[tw], kw[tw], i_in_chunk[tw]] += vals[tw, j]
        in_chunk = valid & (kw >= WIN)
        tc = t[in_chunk]
        kc = kw[tc] - WIN
        assert (kc < CORR).all()
        wts[c_of_t[tc], WIN + kc, i_in_chunk[tc]] += vals[tc, j]
    return wts, basis


def _fold_corr(wts_seg):
    """Fold each chunk's within-chunk correction into the weights of its
    in-segment readers so the ring stores *uncorrected* columns."""
    wts_seg = wts_seg.copy()
    n = wts_seg.shape[0]
    blocks = wts_seg.reshape(n, 5, CHUNK, CHUNK)
    corr_active = np.abs(blocks[:, 4]).reshape(n, -1).max(-1) > 0
    for w in range(n):
        if not corr_active[w]:
            continue
        corrT = blocks[w, 4]
        for r in range(w + 1, min(w + 5, n)):
            g = w - r + 4
            blk = blocks[r, g]
            blk[0:CORR] -= corrT[0:CORR, CORR:] @ blk[CORR:]
    return wts_seg


def _seg_basis(seg_wts_neg):
    """Initial-window footprint (in samples before segment start) actually
    read by each segment's first chunks, from the folded weight blocks."""
    out = []
    for w in seg_wts_neg:
        blocks = w.reshape(CPC, 5, CHUNK, CHUNK)
        b = 0
        for r in range(4):
            for g in range(4 - r):
                blk = blocks[r, g]
                nz = np.nonzero(np.abs(blk).max(axis=1) > 0)[0]
                if nz.size:
                    # window coord 128g+p of chunk r = sample
                    # seg_start + 128r + (128g+p) - 512
                    rel = 128 * g + int(nz.min()) - 512 + 128 * r
                    b = max(b, -rel)
        out.append(b)
    return np.array(out, np.int64)


def _basis_ring0(basis_j, nr_q, g0_q):
    """Initial window columns for one segment: basis b is a unit at window
    position (WIN-basis_j)+b; only window cols g0_q..3 are materialized."""
    r0 = np.zeros((CHUNK, 4 - g0_q, nr_q), np.float16)
    for b in range(basis_j):
        p = (WIN - basis_j) + b
        r0[p % CHUNK, p // CHUNK - g0_q, b] = 1.0
    return r0


# ----------------------------------------------------------------------------
# plan construction (shared across cores; SPMD program)
# ----------------------------------------------------------------------------

def _block_act(seg_wts_neg):
    return np.stack([
        np.abs(w.reshape(CPC, 5, -1)).max(-1) > 0 for w in seg_wts_neg
    ])  # [SEGS, CPC, 5]


def _assign_segments(seg_wts_neg, basis_j):
    """Assign the 32 segments to the (core, chain) grid, minimizing the
    exact PE cost  sum_q sum_r union_blocks(q,r) * nr_q  (matmul free-dim
    work).  Returns assign[s, q] = segment id."""
    act = _block_act(seg_wts_neg)[:, :, :4]  # [SEGS, CPC, 4]

    order = np.argsort(basis_j, kind="stable")
    groups = [order[8 * q: 8 * q + 8].tolist() for q in range(CH)]

    def gcost(g):
        nb = act[list(g)].any(0).sum()
        nr = _nr_of(int(basis_j[list(g)].max()))[0]
        return float(nb) * nr

    # simulated annealing on the exact PE-work objective
    import math
    import random
    rng = random.Random(0)
    cost = sum(gcost(g) for g in groups)
    best = ([list(g) for g in groups], cost)
    iters = 120000
    for it in range(iters):
        T = 2000.0 * (1.0 / 2000.0) ** (it / iters)
        a, b = rng.randrange(CH), rng.randrange(CH)
        if a == b:
            continue
        i, j = rng.randrange(N_CORES), rng.randrange(N_CORES)
        ca0, cb0 = gcost(groups[a]), gcost(groups[b])
        groups[a][i], groups[b][j] = groups[b][j], groups[a][i]
        ca, cb = gcost(groups[a]), gcost(groups[b])
        d = (ca + cb) - (ca0 + cb0)
        if d <= 0 or rng.random() < math.exp(-d / T):
            cost += d
            if cost < best[1]:
                best = ([list(g) for g in groups], cost)
        else:
            groups[a][i], groups[b][j] = groups[b][j], groups[a][i]
    groups = best[0]
    # segment 0 carries the excitation; the program injects x only at
    # chain-0 slots, so its group must be chain 0
    q0 = next(q for q in range(CH) if 0 in groups[q])
    groups[0], groups[q0] = groups[q0], groups[0]
    assign = np.zeros((N_CORES, CH), np.int64)
    for q in range(CH):
        for s in range(N_CORES):
            assign[s, q] = groups[q][s]
    return assign


def _make_plans(seg_wts_neg, assign):
    """Per-slot (q, r) union plans across cores.

    Returns plans[q][r] = wblocks tuple, and corr_y[q][r] = bool (host
    fixup positions)."""
    act = _block_act(seg_wts_neg)
    plans = []
    corr_y = []
    for q in range(CH):
        segs = [assign[s, q] for s in range(N_CORES)]
        u = act[segs].any(0)  # [CPC, 5]
        pq = []
        cq = []
        for r in range(CPC):
            wb = tuple(g for g in range(4) if u[r, g])
            if not wb:
                wb = (3,)
            pq.append(wb)
            cq.append(bool(u[r, 4]))
        plans.append(pq)
        corr_y.append(cq)
    return plans, corr_y


def _pack_weights(seg_wts_neg, plans, assign, s):
    """Pack core s's weight blocks, round-major, partition-major fp16
    layout [128, TOT, 128]."""
    cols = []
    for r in range(CPC):
        for q in range(CH):
            wb = plans[q][r]
            blocks = seg_wts_neg[assign[s, q]].reshape(CPC, 5, CHUNK, CHUNK)
            cols.append(blocks[r, list(wb)])  # [nb, 128, 128]
    flat = np.concatenate(cols, 0)            # [TOT, 128k, 128m]
    return np.ascontiguousarray(
        flat.transpose(1, 0, 2)).astype(np.float16)  # [128, TOT, 128]


def _plan_key(plans):
    return tuple(tuple(pq) for pq in plans)


# ----------------------------------------------------------------------------
# prog TOPS: per-segment transfer operators
# ----------------------------------------------------------------------------

def _build_tops_nc(plans, tot_blocks, chain_basis):
    key = ("T3", _plan_key(plans), tot_blocks, tuple(chain_basis))
    if key in _NC_CACHE:
        return _NC_CACHE[key]
    chain_nr = []
    chain_pidx = []
    chain_g0 = []
    for q in range(CH):
        nr, pidx = _nr_of(int(chain_basis[q]))
        chain_nr.append(nr)
        chain_pidx.append(pidx)
        chain_g0.append((WIN - int(chain_basis[q])) // CHUNK)

    nc = bacc.Bacc("TRN2", target_bir_lowering=False, debug=False,
                   num_devices=N_CORES, enable_partition_id=False)
    wts = nc.dram_tensor("wts", [CHUNK, tot_blocks, CHUNK], F16,
                         kind="ExternalInput")
    xin = nc.dram_tensor("xin", [CHUNK, CH * CPC], F32, kind="ExternalInput")
    ring0 = [
        nc.dram_tensor(f"ring0_{q}", [CHUNK, 4 - chain_g0[q], chain_nr[q]],
                       F16, kind="ExternalInput")
        for q in range(CH)
    ]
    tout = [
        nc.dram_tensor(f"tout_{q}", [CHUNK, 4, chain_nr[q]], F16,
                       kind="ExternalOutput")
        for q in range(CH)
    ]

    with tile.TileContext(nc) as tc:
        with (
            tc.tile_pool(name="state", bufs=1) as state,
            tc.tile_pool(name="wpool", bufs=CPC) as wpool,
            tc.tile_pool(name="psum", bufs=8, space="PSUM") as ppool,
        ):
            ring = [state.tile([CHUNK, RING, chain_nr[q]], F16,
                               name=f"ring{q}")
                    for q in range(CH)]
            xin_sb = state.tile([CHUNK, CH * CPC], F32)
            # All DMA issues go to SP/ACT only (Pool's DIRECT2D path is
            # ~1.4us serialized plus a long final DRAIN; DVE can't issue).
            # Ramp-critical issues alternate between the two queues.
            for q in range(CH):
                eng = nc.sync if q % 2 == 0 else nc.scalar
                eng.dma_start(ring[q][:, 4 + chain_g0[q]: 8, :],
                              ring0[q][:])
            nc.sync.dma_start(xin_sb[:], xin[:])

            # pre-issue the full weight stream (bufs=CPC keeps every round
            # resident in SBUF, decoupling DMA from the compute rate)
            wtiles = []
            woff = 0
            for r in range(CPC):
                nbr = sum(len(plans[q][r]) for q in range(CH))
                wtile = wpool.tile([CHUNK, nbr, CHUNK], F16, tag="w")
                if r == 0:
                    so = 0
                    for q in range(CH):
                        nb_q = len(plans[q][0])
                        eng = nc.sync if q % 2 == 0 else nc.scalar
                        eng.dma_start(
                            wtile[:, so: so + nb_q, :],
                            wts[:, woff + so: woff + so + nb_q, :])
                        so += nb_q
                else:
                    eng = nc.sync if r % 2 == 0 else nc.scalar
                    eng.dma_start(
                        wtile[:, 0:nbr, :], wts[:, woff: woff + nbr, :])
                woff += nbr
                wtiles.append(wtile)

            for r in range(CPC):
                wtile = wtiles[r]
                soff = 0
                for q in range(CH):
                    wb = plans[q][r]
                    nr = chain_nr[q]
                    rc = r % RING
                    psum = ppool.tile([CHUNK, nr], F32, tag=f"acc{q}",
                                      bufs=2)
                    for i, g in enumerate(wb):
                        col = (r + 4 + g) % RING
                        nc.tensor.matmul(
                            psum[:],
                            wtile[:, soff + i, :],
                            ring[q][:, col, :],
                            start=(i == 0),
                            stop=(i == len(wb) - 1),
                        )
                    # serial ring update (weights pre-negated: col = psum + x)
                    # split across ACT / DVE (Pool cannot access PSUM)
                    c1 = (int(0.55 * nr) // 2) * 2
                    nc.scalar.copy(ring[q][:, rc, 0:c1], psum[:, 0:c1])
                    nc.vector.tensor_copy(ring[q][:, rc, c1:nr],
                                          psum[:, c1:nr])
                    if q == 0 and r < 4:
                        slot = r * CH + q
                        pidx = chain_pidx[0]
                        nc.vector.tensor_add(
                            ring[0][:, rc, pidx: pidx + 1],
                            ring[0][:, rc, pidx: pidx + 1],
                            xin_sb[:, slot: slot + 1],
                        )
                    soff += len(wb)
                    # stream the transfer operator (last 4 ring cols) out in
                    # column pairs as soon as they are final
                    if r in (CPC - 3, CPC - 1):
                        base = (r - 1) % RING
                        eng = nc.sync if q % 2 == 0 else nc.scalar
                        eng.dma_start(
                            tout[q][:, (r - (CPC - 4)) - 1:
                                    (r - (CPC - 4)) + 1, :],
                            ring[q][:, base: base + 2, :],
                        )
    nc.compile()
    _NC_CACHE[key] = nc
    return nc


# ----------------------------------------------------------------------------
# prog SOLVE: final pass with known initial windows, single RHS
# ----------------------------------------------------------------------------

def _build_solve_nc(plans, tot_blocks):
    key = ("S3", _plan_key(plans), tot_blocks)
    if key in _NC_CACHE:
        return _NC_CACHE[key]
    NWC = 2 * (4 + CPC)  # fp16 col pairs: data at even cols (4B aligned)

    nc = bacc.Bacc("TRN2", target_bir_lowering=False, debug=False,
                   num_devices=N_CORES, enable_partition_id=False)
    wts = nc.dram_tensor("wts", [CHUNK, tot_blocks, CHUNK], F16,
                         kind="ExternalInput")
    xin = nc.dram_tensor("xin", [CHUNK, CH * CPC], F32, kind="ExternalInput")
    win0 = nc.dram_tensor("win0", [CHUNK, CH, 4], F16, kind="ExternalInput")
    yout = nc.dram_tensor("yout", [CHUNK, CH, CPC], F16,
                          kind="ExternalOutput")

    with tile.TileContext(nc) as tc:
        with (
            tc.tile_pool(name="state", bufs=1) as state,
            tc.tile_pool(name="wpool", bufs=CPC) as wpool,
            tc.tile_pool(name="psum", bufs=8, space="PSUM") as ppool,
        ):
            ywin = state.tile([CHUNK, CH, NWC], F16)
            xin_sb = state.tile([CHUNK, CH * CPC], F32)
            for q in range(CH):
                eng = nc.sync if q % 2 == 0 else nc.scalar
                eng.dma_start(ywin[:, q, 0:8:2], win0[:, q, :])
            nc.sync.dma_start(xin_sb[:], xin[:])

            wtiles = []
            woff = 0
            for r in range(CPC):
                nbr = sum(len(plans[q][r]) for q in range(CH))
                wtile = wpool.tile([CHUNK, nbr, CHUNK], F16, tag="w")
                if r == 0:
                    so = 0
                    for q in range(CH):
                        nb_q = len(plans[q][0])
                        eng = nc.sync if q % 2 == 0 else nc.scalar
                        eng.dma_start(
                            wtile[:, so: so + nb_q, :],
                            wts[:, woff + so: woff + so + nb_q, :])
                        so += nb_q
                else:
                    eng = nc.sync if r % 2 == 0 else nc.scalar
                    eng.dma_start(
                        wtile[:, 0:nbr, :], wts[:, woff: woff + nbr, :])
                woff += nbr
                wtiles.append(wtile)

            for r in range(CPC):
                wtile = wtiles[r]
                soff = 0
                for q in range(CH):
                    wb = plans[q][r]
                    psum = ppool.tile([CHUNK, 1], F32, tag=f"acc{q}",
                                      bufs=2)
                    for i, g in enumerate(wb):
                        col = 2 * (r + g)  # = 2*(4 + (r-4+g))
                        nc.tensor.matmul(
                            psum[:],
                            wtile[:, soff + i, :],
                            ywin[:, q, col: col + 1],
                            start=(i == 0),
                            stop=(i == len(wb) - 1),
                        )
                    slot = r * CH + q
                    if q == 0 and r < 4:
                        # only these slots can carry excitation input
                        nc.vector.scalar_tensor_tensor(
                            out=ywin[:, q, 2 * (4 + r): 2 * (4 + r) + 1],
                            in0=psum[:], scalar=1.0,
                            in1=xin_sb[:, slot: slot + 1],
                            op0=mybir.AluOpType.mult,
                            op1=mybir.AluOpType.add,
                        )
                    elif slot % 2 == 0:
                        nc.scalar.copy(
                            ywin[:, q, 2 * (4 + r): 2 * (4 + r) + 1],
                            psum[:])
                    else:
                        nc.vector.tensor_copy(
                            ywin[:, q, 2 * (4 + r): 2 * (4 + r) + 1],
                            psum[:])
                    soff += len(wb)
            for q in range(CH):
                eng = nc.sync if q % 2 == 0 else nc.scalar
                eng.dma_start(yout[:, q, :],
                              ywin[:, q, 8: 8 + 2 * CPC: 2])
    nc.compile()
    _NC_CACHE[key] = nc
    return nc


# ----------------------------------------------------------------------------
# host orchestration
# ----------------------------------------------------------------------------

def _run(nc, in_maps, tag):
    trace = bool(int(os.environ.get("DIFFKS_TRACE", "0")))
    kw = {}
    tcs = os.environ.get("DIFFKS_TRACE_CORES", "")
    if trace and tcs:
        kw["trace_cores"] = [int(x) for x in tcs.split(",")]
    res = run_bass_kernel_spmd(
        nc, in_maps, core_ids=list(range(len(in_maps))), trace=trace, **kw
    )
    LAST_RESULTS[tag] = res
    return res.results


def kernel(delay_len_frames, raw_coeff_frames, excitation, n_samples):
    n = int(n_samples)
    assert n == N_SAMPLES, f"kernel hardcoded for {N_SAMPLES}, got {n}"
    LAST_RESULTS.clear()

    vals, z_l, x = _preprocess(delay_len_frames, raw_coeff_frames,
                               excitation, n)
    wts, basis = _build_wts(vals, z_l, n)
    n_chunks = n // CHUNK
    assert n_chunks == SEGS * CPC
    xin_cols = np.ascontiguousarray(x.reshape(n_chunks, CHUNK).T)  # [128, nc]

    # fold corrections, then negate everything (update becomes plain copy)
    seg_wts_neg = [-_fold_corr(wts[j * CPC:(j + 1) * CPC])
                   for j in range(SEGS)]
    basis_j = _seg_basis(seg_wts_neg)
    assign = _assign_segments(seg_wts_neg, basis_j)
    inv = {int(assign[s, q]): (s, q)
           for s in range(N_CORES) for q in range(CH)}
    plans, corr_y = _make_plans(seg_wts_neg, assign)
    tot_blocks = sum(len(plans[q][r]) for r in range(CPC) for q in range(CH))
    chain_basis = [int(max(basis_j[assign[s, q]] for s in range(N_CORES)))
                   for q in range(CH)]
    chain_nr = []
    chain_pidx = []
    chain_g0 = []
    for q in range(CH):
        nr, pidx = _nr_of(chain_basis[q])
        chain_nr.append(nr)
        chain_pidx.append(pidx)
        chain_g0.append((WIN - chain_basis[q]) // CHUNK)

    ncT = _build_tops_nc(plans, tot_blocks, chain_basis)
    packed_wts = [_pack_weights(seg_wts_neg, plans, assign, s)
                  for s in range(N_CORES)]
    in_maps = []
    for s in range(N_CORES):
        xin = np.zeros((CHUNK, CH * CPC), np.float32)
        for r in range(CPC):
            for q in range(CH):
                gchunk = int(assign[s, q]) * CPC + r
                xin[:, r * CH + q] = xin_cols[:, gchunk]
        im = {"wts": packed_wts[s], "xin": xin}
        for q in range(CH):
            im[f"ring0_{q}"] = _basis_ring0(
                int(basis_j[assign[s, q]]), chain_nr[q], chain_g0[q])
        in_maps.append(im)
    outsT = _run(ncT, in_maps, "tops")

    # host combine: apply correction fixups to each segment's transfer
    # operator, then chain them (fp32) to get every initial window
    wins = [np.zeros(WIN, np.float32)]
    for j in range(SEGS):
        s, q = inv[j]
        T = outsT[s][f"tout_{q}"].astype(np.float32)  # [128, 4, nr_q]
        blocks = seg_wts_neg[j].reshape(CPC, 5, CHUNK, CHUNK)
        for k in range(4):
            Lc = blocks[CPC - 4 + k, 4][0:CORR]       # negated lhsT [64, 128]
            if np.any(Lc):
                fix = Lc.T @ T[0:CORR, k, :]          # [128, nr]
                T[CORR:, k, :] += fix[CORR:]
        T = T.transpose(1, 0, 2).reshape(WIN, chain_nr[q])
        bj = int(basis_j[j])
        w_next = T[:, :bj] @ wins[j][WIN - bj:] + T[:, chain_pidx[q]]
        wins.append(w_next.astype(np.float32))

    # final pass: known windows, single RHS
    ncS = _build_solve_nc(plans, tot_blocks)
    in_maps = []
    for s in range(N_CORES):
        xin = np.zeros((CHUNK, CH * CPC), np.float32)
        for r in range(CPC):
            for q in range(CH):
                gchunk = int(assign[s, q]) * CPC + r
                xin[:, r * CH + q] = xin_cols[:, gchunk]
        w0 = np.zeros((CHUNK, CH, 4), np.float16)
        for q in range(CH):
            j = int(assign[s, q])
            w0[:, q, :] = wins[j].reshape(4, CHUNK).T.astype(np.float16)
        in_maps.append({"wts": packed_wts[s], "xin": xin, "win0": w0})
    outsS = _run(ncS, in_maps, "solve")

    y = np.zeros(n, np.float32)
    for s in range(N_CORES):
        yo = np.array(outsS[s]["yout"]).astype(np.float32)  # [128, CH, CPC]
        for q in range(CH):
            j = int(assign[s, q])
            blocks = seg_wts_neg[j].reshape(CPC, 5, CHUNK, CHUNK)
            for r in range(CPC):
                col = yo[:, q, r]
                Lc = blocks[r, 4][0:CORR]           # negated lhsT [64, 128]
                if np.any(Lc):
                    fix = Lc.T @ col[0:CORR]
                    col = col.copy()
                    col[CORR:] += fix[CORR:]
                gchunk = j * CPC + r
                y[gchunk * CHUNK:(gchunk + 1) * CHUNK] = col
    return y.astype(np.float32)


# revision 21
# speedup vs baseline: 2.1392x; 1.9967x over previous
"""Trainium2 Bass kernel for nn_DiffKS (differentiable Karplus-Strong).

Structure of the computation:
  y[t] = x[t] - sum_{j=0..5} vals[t,j] * y[t - 1 - z_l[t] - j]
with vals / z_l derived from spline-interpolated delay & coefficient
trajectories.  The feedback lag (1 + z_l + j) is always >= ~93 samples, so
128-sample chunks can be computed as dense banded matmuls against a
512-sample window of past output plus a small within-chunk correction.

Parallel structure (v3):
  - 32 time segments of 2048 samples; each of the 8 cores runs 4 segments
    as INDEPENDENT interleaved chunk-chains (4 chains x 16 rounds).
  - prog TOPS: each chain propagates basis+1 right-hand sides (unit
    initial-window columns + one particular column) through its segment's
    chunked recurrence.  The RHS count is PER-CHAIN (the max initial-window
    footprint over that chain's 8 segments, ~170-390) instead of the global
    max lag (~430).  Only the final 4 ring columns (the segment transfer
    operator T) go to DRAM -- no full response operator streaming.
  - combine (host, tiny): chain the 32 transfer operators to get every
    segment's true initial window.
  - prog SOLVE: re-runs the chunked recurrence with the now-known initial
    windows and a single RHS per segment (cheap N=1 matmuls), writing the
    actual output samples.  Reuses the identical packed weight stream.
  - host: within-chunk correction fix-ups + reorder.

Weights are pre-negated on the host so the serial ring update is a plain
PSUM->SBUF copy (split across the ACT/DVE/Pool engines).  DMA descriptors
are issued round-robin from all four sequencers (the v2 kernel saturated
the Sync sequencer with 565ns-per-issue DMA configs).
"""

import os
import numpy as np

import concourse.bacc as bacc
import concourse.tile as tile
import concourse.mybir as mybir
from concourse.bass_utils import run_bass_kernel_spmd


def _ensure_ntff_hook():
    """The agent image's `antenv` stub lacks `axon_hooks`, which
    `run_bass_kernel_spmd(trace=True)` needs under axon for NTFF capture."""
    try:
        from antenv.axon_hooks import get_axon_ntff_profile_hook  # noqa: F401
        return
    except ImportError:
        pass
    import contextlib
    import ctypes
    import sys
    import types

    so_path = "/opt/axon/libaxon_pjrt.so"
    if not os.path.exists(so_path):
        return
    lib = ctypes.CDLL(so_path)
    if not hasattr(lib, "axon_start_nrt_profile"):
        return
    lib.axon_start_nrt_profile.argtypes = [
        ctypes.POINTER(ctypes.c_int64), ctypes.c_size_t]
    lib.axon_start_nrt_profile.restype = ctypes.c_int64
    lib.axon_stop_nrt_profile.argtypes = [ctypes.c_char_p]
    lib.axon_stop_nrt_profile.restype = ctypes.c_int64

    @contextlib.contextmanager
    def _hook(output_dir, device_ids):
        import jax
        jax.devices()
        if device_ids:
            ids = (ctypes.c_int64 * len(device_ids))(*device_ids)
            rc = lib.axon_start_nrt_profile(ids, len(device_ids))
        else:
            rc = lib.axon_start_nrt_profile(None, 0)
        if rc != 0:
            raise RuntimeError(f"axon_start_nrt_profile rc={rc}")
        try:
            yield
        finally:
            n = lib.axon_stop_nrt_profile(str(output_dir).encode())
            if n <= 0:
                print(f"ntff profile: {n} file(s) written to {output_dir}",
                      file=sys.stderr)

    mod = types.ModuleType("antenv.axon_hooks")
    mod._hook = _hook
    mod.get_axon_ntff_profile_hook = lambda: _hook
    mod.set_axon_ntff_profile_hook = lambda h: setattr(mod, "_hook", h)
    import antenv
    antenv.axon_hooks = mod
    sys.modules["antenv.axon_hooks"] = mod


_ensure_ntff_hook()

F32 = mybir.dt.float32
F16 = mybir.dt.float16

N_SAMPLES = 65536
N_FRAMES = 64
L_ORDER = 5
CHUNK = 128
WIN = 512            # window length the chunk matmuls see (4 ring cols)
RING = 8             # ring columns per chain in SBUF
CORR = 64            # within-chunk correction width (needs z_l >= 63)
N_CORES = 8
CH = 4               # independent chains (segments) per core
CPC = 16             # chunks (rounds) per chain
SEGS = N_CORES * CH  # 32 segments of 2048 samples

# filled by kernel() with per-phase profiling results for the test harness
LAST_RESULTS = {}

_NC_CACHE = {}


# device rhs layout: basis columns [0, basis); one zero pad column; the
# particular column at PIDX (4-byte aligned for the fp16 column update);
# one trailing pad so the total width is even.
def _nr_of(basis):
    pidx = basis + 1 + (basis + 1) % 2
    return pidx + 2 - (basis + 1) % 2, pidx


def _seg_of(s, q):
    """Segment index handled by core s, chain q."""
    return s + N_CORES * q


# ----------------------------------------------------------------------------
# host-side preprocessing
# ----------------------------------------------------------------------------

_SPLINE_CACHE = {}


def _spline_matrix(n_in, n_out):
    """Static [n_out, n_in] natural-cubic-spline interpolation matrix."""
    key = (n_in, n_out)
    if key in _SPLINE_CACHE:
        return _SPLINE_CACHE[key]
    t_in = np.linspace(0.0, 1.0, n_in)
    t_out = np.linspace(0.0, 1.0, n_out)
    n = n_in
    h = t_in[1:] - t_in[:-1]
    R = np.zeros((n - 2, n))
    for i in range(n - 2):
        R[i, i] += 6.0 / h[i]
        R[i, i + 1] += -6.0 / h[i] - 6.0 / h[i + 1]
        R[i, i + 2] += 6.0 / h[i + 1]
    A = (
        np.diag(2.0 * (h[:-1] + h[1:]))
        + np.diag(h[1:-1], 1)
        + np.diag(h[1:-1], -1)
    )
    M = np.zeros((n, n))
    M[1:-1] = np.linalg.solve(A, R)
    idx = np.clip(np.searchsorted(t_in, t_out, side="right") - 1, 0, n - 2)
    dt = t_out - t_in[idx]
    S = np.zeros((n_out, n))
    eye = np.eye(n)
    for r in range(n_out):
        i = idx[r]
        b = (eye[i + 1] - eye[i]) / h[i] - h[i] * (2.0 * M[i] + M[i + 1]) / 6.0
        c = M[i] / 2.0
        d = (M[i + 1] - M[i]) / (6.0 * h[i])
        S[r] = eye[i] + b * dt[r] + c * dt[r] ** 2 + d * dt[r] ** 3
    S = S.astype(np.float32)
    _SPLINE_CACHE[key] = S
    return S


def _preprocess(delay, raw, exc, n_samples):
    sig = 1.0 / (1.0 + np.exp(-np.asarray(raw, np.float32)))
    coeff = sig / sig.sum(-1, keepdims=True)
    S = _spline_matrix(N_FRAMES, n_samples)
    delay_interp = S @ np.asarray(delay, np.float32)
    coeff_interp = S @ coeff
    z_l = np.floor(delay_interp).astype(np.int32)
    alfa = (delay_interp - z_l).astype(np.float32)
    b = coeff_interp
    v0 = -(1.0 - alfa) * b[:, 0]
    vmid = -(alfa[:, None] * b[:, : L_ORDER - 1]
             + (1.0 - alfa)[:, None] * b[:, 1:L_ORDER])
    vL = -alfa * b[:, -1]
    vals = np.concatenate([v0[:, None], vmid, vL[:, None]], 1).astype(np.float32)
    x = np.zeros(n_samples, np.float32)
    exc = np.asarray(exc, np.float32)
    x[: exc.shape[0]] = exc
    return vals, z_l, x


def _build_wts(vals, z_l, n_samples):
    """Dense per-chunk matmul weights in lhsT layout.

    wts[c, 128g + p, m] = W[c][m, 128g + p]   (g = 0..3, window blocks)
    wts[c, 512 + p, m]  = L[c][m, p]          (p < 64, correction block)
    """
    n_chunks = n_samples // CHUNK
    t = np.arange(n_samples)
    lag = 1 + z_l[:, None] + np.arange(6)[None, :]
    assert (lag[:, 0] >= CORR).all(), "delay too small for correction width"
    basis = int(lag.max())
    assert basis <= WIN - CORR, "delay too large for window"
    src = t[:, None] - lag
    i_in_chunk = t % CHUNK
    k_win = WIN + i_in_chunk[:, None] - lag
    wts = np.zeros((n_chunks, 5 * CHUNK, CHUNK), np.float32)
    c_of_t = t // CHUNK
    for j in range(6):
        valid = src[:, j] >= 0
        kw = k_win[:, j]
        in_window = valid & (kw < WIN)
        tw = t[in_window]
        wts[c_of_t[tw], kw[tw], i_in_chunk[tw]] += vals[tw, j]
        in_chunk = valid & (kw >= WIN)
        tc = t[in_chunk]
        kc = kw[tc] - WIN
        assert (kc < CORR).all()
        wts[c_of_t[tc], WIN + kc, i_in_chunk[tc]] += vals[tc, j]
    return wts, basis


def _fold_corr(wts_seg):
    """Fold each chunk's within-chunk correction into the weights of its
    in-segment readers so the ring stores *uncorrected* columns."""
    wts_seg = wts_seg.copy()
    n = wts_seg.shape[0]
    blocks = wts_seg.reshape(n, 5, CHUNK, CHUNK)
    corr_active = np.abs(blocks[:, 4]).reshape(n, -1).max(-1) > 0
    for w in range(n):
        if not corr_active[w]:
            continue
        corrT = blocks[w, 4]
        for r in range(w + 1, min(w + 5, n)):
            g = w - r + 4
            blk = blocks[r, g]
            blk[0:CORR] -= corrT[0:CORR, CORR:] @ blk[CORR:]
    return wts_seg


def _seg_basis(seg_wts_neg):
    """Initial-window footprint (in samples before segment start) actually
    read by each segment's first chunks, from the folded weight blocks."""
    out = []
    for w in seg_wts_neg:
        blocks = w.reshape(CPC, 5, CHUNK, CHUNK)
        b = 0
        for r in range(4):
            for g in range(4 - r):
                blk = blocks[r, g]
                nz = np.nonzero(np.abs(blk).max(axis=1) > 0)[0]
                if nz.size:
                    # window coord 128g+p of chunk r = sample
                    # seg_start + 128r + (128g+p) - 512
                    rel = 128 * g + int(nz.min()) - 512 + 128 * r
                    b = max(b, -rel)
        out.append(b)
    return np.array(out, np.int64)


def _basis_ring0(basis_j, nr_q, g0_q):
    """Initial window columns for one segment: basis b is a unit at window
    position (WIN-basis_j)+b; only window cols g0_q..3 are materialized."""
    r0 = np.zeros((CHUNK, 4 - g0_q, nr_q), np.float16)
    for b in range(basis_j):
        p = (WIN - basis_j) + b
        r0[p % CHUNK, p // CHUNK - g0_q, b] = 1.0
    return r0


# ----------------------------------------------------------------------------
# plan construction (shared across cores; SPMD program)
# ----------------------------------------------------------------------------

def _block_act(seg_wts_neg):
    return np.stack([
        np.abs(w.reshape(CPC, 5, -1)).max(-1) > 0 for w in seg_wts_neg
    ])  # [SEGS, CPC, 5]


def _assign_segments(seg_wts_neg, basis_j):
    """Assign the 32 segments to the (core, chain) grid, minimizing the
    exact PE cost  sum_q sum_r union_blocks(q,r) * nr_q  (matmul free-dim
    work).  Returns assign[s, q] = segment id."""
    act = _block_act(seg_wts_neg)[:, :, :4]  # [SEGS, CPC, 4]

    order = np.argsort(basis_j, kind="stable")
    groups = [order[8 * q: 8 * q + 8].tolist() for q in range(CH)]

    def gcost(g):
        nb = act[list(g)].any(0).sum()
        nr = _nr_of(int(basis_j[list(g)].max()))[0]
        return float(nb) * nr

    # simulated annealing on the exact PE-work objective
    import math
    import random
    rng = random.Random(0)
    cost = sum(gcost(g) for g in groups)
    best = ([list(g) for g in groups], cost)
    iters = 120000
    for it in range(iters):
        T = 2000.0 * (1.0 / 2000.0) ** (it / iters)
        a, b = rng.randrange(CH), rng.randrange(CH)
        if a == b:
            continue
        i, j = rng.randrange(N_CORES), rng.randrange(N_CORES)
        ca0, cb0 = gcost(groups[a]), gcost(groups[b])
        groups[a][i], groups[b][j] = groups[b][j], groups[a][i]
        ca, cb = gcost(groups[a]), gcost(groups[b])
        d = (ca + cb) - (ca0 + cb0)
        if d <= 0 or rng.random() < math.exp(-d / T):
            cost += d
            if cost < best[1]:
                best = ([list(g) for g in groups], cost)
        else:
            groups[a][i], groups[b][j] = groups[b][j], groups[a][i]
    groups = best[0]
    # segment 0 carries the excitation; the program injects x only at
    # chain-0 slots, so its group must be chain 0
    q0 = next(q for q in range(CH) if 0 in groups[q])
    groups[0], groups[q0] = groups[q0], groups[0]
    assign = np.zeros((N_CORES, CH), np.int64)
    for q in range(CH):
        for s in range(N_CORES):
            assign[s, q] = groups[q][s]
    return assign


def _make_plans(seg_wts_neg, assign):
    """Per-slot (q, r) union plans across cores.

    Returns plans[q][r] = wblocks tuple, and corr_y[q][r] = bool (host
    fixup positions)."""
    act = _block_act(seg_wts_neg)
    plans = []
    corr_y = []
    for q in range(CH):
        segs = [assign[s, q] for s in range(N_CORES)]
        u = act[segs].any(0)  # [CPC, 5]
        pq = []
        cq = []
        for r in range(CPC):
            wb = tuple(g for g in range(4) if u[r, g])
            if not wb:
                wb = (3,)
            pq.append(wb)
            cq.append(bool(u[r, 4]))
        plans.append(pq)
        corr_y.append(cq)
    return plans, corr_y


def _pack_weights(seg_wts_neg, plans, assign, s):
    """Pack core s's weight blocks, round-major, partition-major fp16
    layout [128, TOT, 128]."""
    cols = []
    for r in range(CPC):
        for q in range(CH):
            wb = plans[q][r]
            blocks = seg_wts_neg[assign[s, q]].reshape(CPC, 5, CHUNK, CHUNK)
            cols.append(blocks[r, list(wb)])  # [nb, 128, 128]
    flat = np.concatenate(cols, 0)            # [TOT, 128k, 128m]
    return np.ascontiguousarray(
        flat.transpose(1, 0, 2)).astype(np.float16)  # [128, TOT, 128]


def _plan_key(plans):
    return tuple(tuple(pq) for pq in plans)


# ----------------------------------------------------------------------------
# prog TOPS: per-segment transfer operators
# ----------------------------------------------------------------------------

def _build_tops_nc(plans, tot_blocks, chain_basis):
    key = ("T3", _plan_key(plans), tot_blocks, tuple(chain_basis))
    if key in _NC_CACHE:
        return _NC_CACHE[key]
    chain_nr = []
    chain_pidx = []
    chain_g0 = []
    for q in range(CH):
        nr, pidx = _nr_of(int(chain_basis[q]))
        chain_nr.append(nr)
        chain_pidx.append(pidx)
        chain_g0.append((WIN - int(chain_basis[q])) // CHUNK)

    nc = bacc.Bacc("TRN2", target_bir_lowering=False, debug=False,
                   num_devices=N_CORES, enable_partition_id=False)
    wts = nc.dram_tensor("wts", [CHUNK, tot_blocks, CHUNK], F16,
                         kind="ExternalInput")
    xin = nc.dram_tensor("xin", [CHUNK, CH * CPC], F32, kind="ExternalInput")
    ring0 = [
        nc.dram_tensor(f"ring0_{q}", [CHUNK, 4 - chain_g0[q], chain_nr[q]],
                       F16, kind="ExternalInput")
        for q in range(CH)
    ]
    hout = [
        nc.dram_tensor(f"hout_{q}", [CHUNK, CPC, chain_nr[q]], F16,
                       kind="ExternalOutput")
        for q in range(CH)
    ]

    with tile.TileContext(nc) as tc:
        with (
            tc.tile_pool(name="state", bufs=1) as state,
            tc.tile_pool(name="wpool", bufs=CPC) as wpool,
            tc.tile_pool(name="psum", bufs=8, space="PSUM") as ppool,
        ):
            ring = [state.tile([CHUNK, RING, chain_nr[q]], F16,
                               name=f"ring{q}")
                    for q in range(CH)]
            xin_sb = state.tile([CHUNK, CH * CPC], F32)
            # All DMA issues go to SP/ACT only (Pool's DIRECT2D path is
            # ~1.4us serialized plus a long final DRAIN; DVE can't issue).
            # Ramp-critical issues alternate between the two queues.
            for q in range(CH):
                eng = nc.sync if q % 2 == 0 else nc.scalar
                eng.dma_start(ring[q][:, 4 + chain_g0[q]: 8, :],
                              ring0[q][:])
            nc.sync.dma_start(xin_sb[:], xin[:])

            # pre-issue the full weight stream (bufs=CPC keeps every round
            # resident in SBUF, decoupling DMA from the compute rate)
            wtiles = []
            woff = 0
            for r in range(CPC):
                nbr = sum(len(plans[q][r]) for q in range(CH))
                wtile = wpool.tile([CHUNK, nbr, CHUNK], F16, tag="w")
                if r == 0:
                    so = 0
                    for q in range(CH):
                        nb_q = len(plans[q][0])
                        eng = nc.sync if q % 2 == 0 else nc.scalar
                        eng.dma_start(
                            wtile[:, so: so + nb_q, :],
                            wts[:, woff + so: woff + so + nb_q, :])
                        so += nb_q
                else:
                    eng = nc.sync if r % 2 == 0 else nc.scalar
                    eng.dma_start(
                        wtile[:, 0:nbr, :], wts[:, woff: woff + nbr, :])
                woff += nbr
                wtiles.append(wtile)

            for r in range(CPC):
                wtile = wtiles[r]
                soff = 0
                for q in range(CH):
                    wb = plans[q][r]
                    nr = chain_nr[q]
                    rc = r % RING
                    psum = ppool.tile([CHUNK, nr], F32, tag=f"acc{q}",
                                      bufs=2)
                    for i, g in enumerate(wb):
                        col = (r + 4 + g) % RING
                        nc.tensor.matmul(
                            psum[:],
                            wtile[:, soff + i, :],
                            ring[q][:, col, :],
                            start=(i == 0),
                            stop=(i == len(wb) - 1),
                        )
                    # serial ring update (weights pre-negated: col = psum + x)
                    # split across ACT / DVE (Pool cannot access PSUM)
                    c1 = (int(0.55 * nr) // 2) * 2
                    nc.scalar.copy(ring[q][:, rc, 0:c1], psum[:, 0:c1])
                    nc.vector.tensor_copy(ring[q][:, rc, c1:nr],
                                          psum[:, c1:nr])
                    if q == 0 and r < 4:
                        slot = r * CH + q
                        pidx = chain_pidx[0]
                        nc.vector.tensor_add(
                            ring[0][:, rc, pidx: pidx + 1],
                            ring[0][:, rc, pidx: pidx + 1],
                            xin_sb[:, slot: slot + 1],
                        )
                    soff += len(wb)
                    # stream the response-operator columns out in 4-round
                    # batches (ring cols (r-3..r)%8 are contiguous)
                    if r % 4 == 3:
                        base = (r - 3) % RING
                        eng = nc.sync if (r + q) % 2 == 0 else nc.scalar
                        eng.dma_start(
                            hout[q][:, r - 3: r + 1, :],
                            ring[q][:, base: base + 4, :],
                        )
    nc.compile()
    _NC_CACHE[key] = nc
    return nc


# ----------------------------------------------------------------------------
# prog SOLVE: final pass with known initial windows, single RHS
# ----------------------------------------------------------------------------

def _build_solve_nc(plans, tot_blocks):
    key = ("S3", _plan_key(plans), tot_blocks)
    if key in _NC_CACHE:
        return _NC_CACHE[key]
    NWC = 2 * (4 + CPC)  # fp16 col pairs: data at even cols (4B aligned)

    nc = bacc.Bacc("TRN2", target_bir_lowering=False, debug=False,
                   num_devices=N_CORES, enable_partition_id=False)
    wts = nc.dram_tensor("wts", [CHUNK, tot_blocks, CHUNK], F16,
                         kind="ExternalInput")
    xin = nc.dram_tensor("xin", [CHUNK, CH * CPC], F32, kind="ExternalInput")
    win0 = nc.dram_tensor("win0", [CHUNK, CH, 4], F16, kind="ExternalInput")
    yout = nc.dram_tensor("yout", [CHUNK, CH, CPC], F16,
                          kind="ExternalOutput")

    with tile.TileContext(nc) as tc:
        with (
            tc.tile_pool(name="state", bufs=1) as state,
            tc.tile_pool(name="wpool", bufs=CPC) as wpool,
            tc.tile_pool(name="psum", bufs=8, space="PSUM") as ppool,
        ):
            ywin = state.tile([CHUNK, CH, NWC], F16)
            xin_sb = state.tile([CHUNK, CH * CPC], F32)
            for q in range(CH):
                eng = nc.sync if q % 2 == 0 else nc.scalar
                eng.dma_start(ywin[:, q, 0:8:2], win0[:, q, :])
            nc.sync.dma_start(xin_sb[:], xin[:])

            wtiles = []
            woff = 0
            for r in range(CPC):
                nbr = sum(len(plans[q][r]) for q in range(CH))
                wtile = wpool.tile([CHUNK, nbr, CHUNK], F16, tag="w")
                if r == 0:
                    so = 0
                    for q in range(CH):
                        nb_q = len(plans[q][0])
                        eng = nc.sync if q % 2 == 0 else nc.scalar
                        eng.dma_start(
                            wtile[:, so: so + nb_q, :],
                            wts[:, woff + so: woff + so + nb_q, :])
                        so += nb_q
                else:
                    eng = nc.sync if r % 2 == 0 else nc.scalar
                    eng.dma_start(
                        wtile[:, 0:nbr, :], wts[:, woff: woff + nbr, :])
                woff += nbr
                wtiles.append(wtile)

            for r in range(CPC):
                wtile = wtiles[r]
                soff = 0
                for q in range(CH):
                    wb = plans[q][r]
                    psum = ppool.tile([CHUNK, 1], F32, tag=f"acc{q}",
                                      bufs=2)
                    for i, g in enumerate(wb):
                        col = 2 * (r + g)  # = 2*(4 + (r-4+g))
                        nc.tensor.matmul(
                            psum[:],
                            wtile[:, soff + i, :],
                            ywin[:, q, col: col + 1],
                            start=(i == 0),
                            stop=(i == len(wb) - 1),
                        )
                    slot = r * CH + q
                    if q == 0 and r < 4:
                        # only these slots can carry excitation input
                        nc.vector.scalar_tensor_tensor(
                            out=ywin[:, q, 2 * (4 + r): 2 * (4 + r) + 1],
                            in0=psum[:], scalar=1.0,
                            in1=xin_sb[:, slot: slot + 1],
                            op0=mybir.AluOpType.mult,
                            op1=mybir.AluOpType.add,
                        )
                    elif slot % 2 == 0:
                        nc.scalar.copy(
                            ywin[:, q, 2 * (4 + r): 2 * (4 + r) + 1],
                            psum[:])
                    else:
                        nc.vector.tensor_copy(
                            ywin[:, q, 2 * (4 + r): 2 * (4 + r) + 1],
                            psum[:])
                    soff += len(wb)
            for q in range(CH):
                eng = nc.sync if q % 2 == 0 else nc.scalar
                eng.dma_start(yout[:, q, :],
                              ywin[:, q, 8: 8 + 2 * CPC: 2])
    nc.compile()
    _NC_CACHE[key] = nc
    return nc


# ----------------------------------------------------------------------------
# host orchestration
# ----------------------------------------------------------------------------

def _run(nc, in_maps, tag):
    trace = bool(int(os.environ.get("DIFFKS_TRACE", "0")))
    kw = {}
    tcs = os.environ.get("DIFFKS_TRACE_CORES", "")
    if trace and tcs:
        kw["trace_cores"] = [int(x) for x in tcs.split(",")]
    res = run_bass_kernel_spmd(
        nc, in_maps, core_ids=list(range(len(in_maps))), trace=trace, **kw
    )
    LAST_RESULTS[tag] = res
    return res.results


def kernel(delay_len_frames, raw_coeff_frames, excitation, n_samples):
    n = int(n_samples)
    assert n == N_SAMPLES, f"kernel hardcoded for {N_SAMPLES}, got {n}"
    LAST_RESULTS.clear()

    vals, z_l, x = _preprocess(delay_len_frames, raw_coeff_frames,
                               excitation, n)
    wts, basis = _build_wts(vals, z_l, n)
    n_chunks = n // CHUNK
    assert n_chunks == SEGS * CPC
    xin_cols = np.ascontiguousarray(x.reshape(n_chunks, CHUNK).T)  # [128, nc]

    # fold corrections, then negate everything (update becomes plain copy)
    seg_wts_neg = [-_fold_corr(wts[j * CPC:(j + 1) * CPC])
                   for j in range(SEGS)]
    basis_j = _seg_basis(seg_wts_neg)
    assign = _assign_segments(seg_wts_neg, basis_j)
    inv = {int(assign[s, q]): (s, q)
           for s in range(N_CORES) for q in range(CH)}
    plans, corr_y = _make_plans(seg_wts_neg, assign)
    tot_blocks = sum(len(plans[q][r]) for r in range(CPC) for q in range(CH))
    chain_basis = [int(max(basis_j[assign[s, q]] for s in range(N_CORES)))
                   for q in range(CH)]
    chain_nr = []
    chain_pidx = []
    chain_g0 = []
    for q in range(CH):
        nr, pidx = _nr_of(chain_basis[q])
        chain_nr.append(nr)
        chain_pidx.append(pidx)
        chain_g0.append((WIN - chain_basis[q]) // CHUNK)

    ncT = _build_tops_nc(plans, tot_blocks, chain_basis)
    packed_wts = [_pack_weights(seg_wts_neg, plans, assign, s)
                  for s in range(N_CORES)]
    in_maps = []
    for s in range(N_CORES):
        xin = np.zeros((CHUNK, CH * CPC), np.float32)
        for r in range(CPC):
            for q in range(CH):
                gchunk = int(assign[s, q]) * CPC + r
                xin[:, r * CH + q] = xin_cols[:, gchunk]
        im = {"wts": packed_wts[s], "xin": xin}
        for q in range(CH):
            im[f"ring0_{q}"] = _basis_ring0(
                int(basis_j[assign[s, q]]), chain_nr[q], chain_g0[q])
        in_maps.append(im)
    outsT = _run(ncT, in_maps, "tops")

    # host combine: apply correction fixups to each segment's transfer
    # operator (the last 4 response columns), then chain them (fp32) to get
    # every segment's true initial window
    Hs = {(s, q): outsT[s][f"hout_{q}"].astype(np.float32)
          for s in range(N_CORES) for q in range(CH)}  # [128, CPC, nr_q]
    wins = [np.zeros(WIN, np.float32)]
    for j in range(SEGS):
        s, q = inv[j]
        T = np.array(Hs[(s, q)][:, CPC - 4: CPC, :])   # [128, 4, nr_q]
        blocks = seg_wts_neg[j].reshape(CPC, 5, CHUNK, CHUNK)
        for k in range(4):
            Lc = blocks[CPC - 4 + k, 4][0:CORR]       # negated lhsT [64, 128]
            if np.any(Lc):
                fix = Lc.T @ T[0:CORR, k, :]          # [128, nr]
                T[CORR:, k, :] += fix[CORR:]
        T = T.transpose(1, 0, 2).reshape(WIN, chain_nr[q])
        bj = int(basis_j[j])
        w_next = T[:, :bj] @ wins[j][WIN - bj:] + T[:, chain_pidx[q]]
        wins.append(w_next.astype(np.float32))

    # host apply: y chunks = H @ [w; 1] per segment, plus within-chunk
    # correction fix-ups
    y = np.zeros(n, np.float32)
    for s in range(N_CORES):
        for q in range(CH):
            j = int(assign[s, q])
            wv = np.zeros(chain_nr[q], np.float32)
            bj = int(basis_j[j])
            wv[:bj] = wins[j][WIN - bj:]
            wv[chain_pidx[q]] = 1.0
            yo = Hs[(s, q)] @ wv                       # [128, CPC]
            blocks = seg_wts_neg[j].reshape(CPC, 5, CHUNK, CHUNK)
            for r in range(CPC):
                col = yo[:, r]
                Lc = blocks[r, 4][0:CORR]           # negated lhsT [64, 128]
                if np.any(Lc):
                    fix = Lc.T @ col[0:CORR]
                    col = col.copy()
                    col[CORR:] += fix[CORR:]
                gchunk = j * CPC + r
                y[gchunk * CHUNK:(gchunk + 1) * CHUNK] = col
    return y.astype(np.float32)


# revision 30
# speedup vs baseline: 2.4731x; 1.1561x over previous
"""Trainium2 Bass kernel for nn_DiffKS (differentiable Karplus-Strong).

Structure of the computation:
  y[t] = x[t] - sum_{j=0..5} vals[t,j] * y[t - 1 - z_l[t] - j]
with vals / z_l derived from spline-interpolated delay & coefficient
trajectories.  The feedback lag (1 + z_l + j) is always >= ~93 samples, so
128-sample chunks can be computed as dense banded matmuls against a
512-sample window of past output plus a small within-chunk correction.

Parallel structure (v3):
  - 32 time segments of 2048 samples; each of the 8 cores runs 4 segments
    as INDEPENDENT interleaved chunk-chains (4 chains x 16 rounds).
  - prog TOPS: each chain propagates basis+1 right-hand sides (unit
    initial-window columns + one particular column) through its segment's
    chunked recurrence.  The RHS count is PER-CHAIN (the max initial-window
    footprint over that chain's 8 segments, ~170-390) instead of the global
    max lag (~430).  Only the final 4 ring columns (the segment transfer
    operator T) go to DRAM -- no full response operator streaming.
  - combine (host, tiny): chain the 32 transfer operators to get every
    segment's true initial window.
  - prog SOLVE: re-runs the chunked recurrence with the now-known initial
    windows and a single RHS per segment (cheap N=1 matmuls), writing the
    actual output samples.  Reuses the identical packed weight stream.
  - host: within-chunk correction fix-ups + reorder.

Weights are pre-negated on the host so the serial ring update is a plain
PSUM->SBUF copy (split across the ACT/DVE/Pool engines).  DMA descriptors
are issued round-robin from all four sequencers (the v2 kernel saturated
the Sync sequencer with 565ns-per-issue DMA configs).
"""

import os
import numpy as np

import concourse.bacc as bacc
import concourse.tile as tile
import concourse.mybir as mybir
from concourse.bass_utils import run_bass_kernel_spmd


def _ensure_ntff_hook():
    """The agent image's `antenv` stub lacks `axon_hooks`, which
    `run_bass_kernel_spmd(trace=True)` needs under axon for NTFF capture."""
    try:
        from antenv.axon_hooks import get_axon_ntff_profile_hook  # noqa: F401
        return
    except ImportError:
        pass
    import contextlib
    import ctypes
    import sys
    import types

    so_path = "/opt/axon/libaxon_pjrt.so"
    if not os.path.exists(so_path):
        return
    lib = ctypes.CDLL(so_path)
    if not hasattr(lib, "axon_start_nrt_profile"):
        return
    lib.axon_start_nrt_profile.argtypes = [
        ctypes.POINTER(ctypes.c_int64), ctypes.c_size_t]
    lib.axon_start_nrt_profile.restype = ctypes.c_int64
    lib.axon_stop_nrt_profile.argtypes = [ctypes.c_char_p]
    lib.axon_stop_nrt_profile.restype = ctypes.c_int64

    @contextlib.contextmanager
    def _hook(output_dir, device_ids):
        import jax
        jax.devices()
        if device_ids:
            ids = (ctypes.c_int64 * len(device_ids))(*device_ids)
            rc = lib.axon_start_nrt_profile(ids, len(device_ids))
        else:
            rc = lib.axon_start_nrt_profile(None, 0)
        if rc != 0:
            raise RuntimeError(f"axon_start_nrt_profile rc={rc}")
        try:
            yield
        finally:
            n = lib.axon_stop_nrt_profile(str(output_dir).encode())
            if n <= 0:
                print(f"ntff profile: {n} file(s) written to {output_dir}",
                      file=sys.stderr)

    mod = types.ModuleType("antenv.axon_hooks")
    mod._hook = _hook
    mod.get_axon_ntff_profile_hook = lambda: _hook
    mod.set_axon_ntff_profile_hook = lambda h: setattr(mod, "_hook", h)
    import antenv
    antenv.axon_hooks = mod
    sys.modules["antenv.axon_hooks"] = mod


_ensure_ntff_hook()

F32 = mybir.dt.float32
F16 = mybir.dt.float16

N_SAMPLES = 65536
N_FRAMES = 64
L_ORDER = 5
CHUNK = 128
WIN = 512            # window length the chunk matmuls see (4 ring cols)
RING = 8             # ring columns per chain in SBUF
CORR = 64            # within-chunk correction width (needs z_l >= 63)
N_CORES = 8
CH = 4               # independent chains (segments) per core
CPC = 16             # chunks (rounds) per chain
SEGS = N_CORES * CH  # 32 segments of 2048 samples

# filled by kernel() with per-phase profiling results for the test harness
LAST_RESULTS = {}

_NC_CACHE = {}


# device rhs layout: basis columns [0, basis); one zero pad column; the
# particular column at PIDX (4-byte aligned for the fp16 column update);
# one trailing pad so the total width is even.
def _nr_of(basis):
    pidx = basis + 1 + (basis + 1) % 2
    return pidx + 2 - (basis + 1) % 2, pidx


def _seg_of(s, q):
    """Segment index handled by core s, chain q."""
    return s + N_CORES * q


# ----------------------------------------------------------------------------
# host-side preprocessing
# ----------------------------------------------------------------------------

_SPLINE_CACHE = {}


def _spline_matrix(n_in, n_out):
    """Static [n_out, n_in] natural-cubic-spline interpolation matrix."""
    key = (n_in, n_out)
    if key in _SPLINE_CACHE:
        return _SPLINE_CACHE[key]
    t_in = np.linspace(0.0, 1.0, n_in)
    t_out = np.linspace(0.0, 1.0, n_out)
    n = n_in
    h = t_in[1:] - t_in[:-1]
    R = np.zeros((n - 2, n))
    for i in range(n - 2):
        R[i, i] += 6.0 / h[i]
        R[i, i + 1] += -6.0 / h[i] - 6.0 / h[i + 1]
        R[i, i + 2] += 6.0 / h[i + 1]
    A = (
        np.diag(2.0 * (h[:-1] + h[1:]))
        + np.diag(h[1:-1], 1)
        + np.diag(h[1:-1], -1)
    )
    M = np.zeros((n, n))
    M[1:-1] = np.linalg.solve(A, R)
    idx = np.clip(np.searchsorted(t_in, t_out, side="right") - 1, 0, n - 2)
    dt = t_out - t_in[idx]
    S = np.zeros((n_out, n))
    eye = np.eye(n)
    for r in range(n_out):
        i = idx[r]
        b = (eye[i + 1] - eye[i]) / h[i] - h[i] * (2.0 * M[i] + M[i + 1]) / 6.0
        c = M[i] / 2.0
        d = (M[i + 1] - M[i]) / (6.0 * h[i])
        S[r] = eye[i] + b * dt[r] + c * dt[r] ** 2 + d * dt[r] ** 3
    S = S.astype(np.float32)
    _SPLINE_CACHE[key] = S
    return S


def _preprocess(delay, raw, exc, n_samples):
    sig = 1.0 / (1.0 + np.exp(-np.asarray(raw, np.float32)))
    coeff = sig / sig.sum(-1, keepdims=True)
    S = _spline_matrix(N_FRAMES, n_samples)
    delay_interp = S @ np.asarray(delay, np.float32)
    coeff_interp = S @ coeff
    z_l = np.floor(delay_interp).astype(np.int32)
    alfa = (delay_interp - z_l).astype(np.float32)
    b = coeff_interp
    v0 = -(1.0 - alfa) * b[:, 0]
    vmid = -(alfa[:, None] * b[:, : L_ORDER - 1]
             + (1.0 - alfa)[:, None] * b[:, 1:L_ORDER])
    vL = -alfa * b[:, -1]
    vals = np.concatenate([v0[:, None], vmid, vL[:, None]], 1).astype(np.float32)
    x = np.zeros(n_samples, np.float32)
    exc = np.asarray(exc, np.float32)
    x[: exc.shape[0]] = exc
    return vals, z_l, x


def _build_wts(vals, z_l, n_samples):
    """Dense per-chunk matmul weights in lhsT layout.

    wts[c, 128g + p, m] = W[c][m, 128g + p]   (g = 0..3, window blocks)
    wts[c, 512 + p, m]  = L[c][m, p]          (p < 64, correction block)
    """
    n_chunks = n_samples // CHUNK
    t = np.arange(n_samples)
    lag = 1 + z_l[:, None] + np.arange(6)[None, :]
    assert (lag[:, 0] >= CORR).all(), "delay too small for correction width"
    basis = int(lag.max())
    assert basis <= WIN - CORR, "delay too large for window"
    src = t[:, None] - lag
    i_in_chunk = t % CHUNK
    k_win = WIN + i_in_chunk[:, None] - lag
    wts = np.zeros((n_chunks, 5 * CHUNK, CHUNK), np.float32)
    c_of_t = t // CHUNK
    for j in range(6):
        valid = src[:, j] >= 0
        kw = k_win[:, j]
        in_window = valid & (kw < WIN)
        tw = t[in_window]
        wts[c_of_t[tw], kw[tw], i_in_chunk[tw]] += vals[tw, j]
        in_chunk = valid & (kw >= WIN)
        tc = t[in_chunk]
        kc = kw[tc] - WIN
        assert (kc < CORR).all()
        wts[c_of_t[tc], WIN + kc, i_in_chunk[tc]] += vals[tc, j]
    return wts, basis


def _fold_corr(wts_seg):
    """Fold each chunk's within-chunk correction into the weights of its
    in-segment readers so the ring stores *uncorrected* columns."""
    wts_seg = wts_seg.copy()
    n = wts_seg.shape[0]
    blocks = wts_seg.reshape(n, 5, CHUNK, CHUNK)
    corr_active = np.abs(blocks[:, 4]).reshape(n, -1).max(-1) > 0
    for w in range(n):
        if not corr_active[w]:
            continue
        corrT = blocks[w, 4]
        for r in range(w + 1, min(w + 5, n)):
            g = w - r + 4
            blk = blocks[r, g]
            blk[0:CORR] -= corrT[0:CORR, CORR:] @ blk[CORR:]
    return wts_seg


def _seg_basis(seg_wts_neg):
    """Initial-window footprint (in samples before segment start) actually
    read by each segment's first chunks, from the folded weight blocks."""
    out = []
    for w in seg_wts_neg:
        blocks = w.reshape(CPC, 5, CHUNK, CHUNK)
        b = 0
        for r in range(4):
            for g in range(4 - r):
                blk = blocks[r, g]
                nz = np.nonzero(np.abs(blk).max(axis=1) > 0)[0]
                if nz.size:
                    # window coord 128g+p of chunk r = sample
                    # seg_start + 128r + (128g+p) - 512
                    rel = 128 * g + int(nz.min()) - 512 + 128 * r
                    b = max(b, -rel)
        out.append(b)
    return np.array(out, np.int64)


# ----------------------------------------------------------------------------
# plan construction (shared across cores; SPMD program)
# ----------------------------------------------------------------------------

def _block_act(seg_wts_neg):
    return np.stack([
        np.abs(w.reshape(CPC, 5, -1)).max(-1) > 0 for w in seg_wts_neg
    ])  # [SEGS, CPC, 5]


def _split_act(seg_wts_neg):
    """Split block activity into in-segment blocks and a pre-segment
    (initial window) indicator.  Block g of round r is pre-segment iff
    r - 4 + g < 0; those reads are folded into host-precomputed C
    matrices applied via one identity matmul."""
    act = _block_act(seg_wts_neg)  # [SEGS, CPC, 5]
    inseg = act[:, :, :4].copy()
    pre = np.zeros((len(seg_wts_neg), CPC), bool)
    for r in range(CPC):
        for g in range(4):
            if r - 4 + g < 0:
                pre[:, r] |= inseg[:, r, g]
                inseg[:, r, g] = False
    return inseg, pre


def _assign_segments(seg_wts_neg, basis_j):
    """Assign the 32 segments to the (core, chain) grid, minimizing the
    exact PE cost  sum_q sum_r union_matmuls(q,r) * nr_q  (matmul free-dim
    work).  Returns assign[s, q] = segment id."""
    inseg, pre = _split_act(seg_wts_neg)

    order = np.argsort(basis_j, kind="stable")
    groups = [order[8 * q: 8 * q + 8].tolist() for q in range(CH)]

    def gcost(g):
        nb = inseg[list(g)].any(0).sum() + pre[list(g)].any(0).sum()
        nr = _nr_of(int(basis_j[list(g)].max()))[0]
        return float(nb) * nr

    # simulated annealing on the exact PE-work objective
    import math
    import random
    rng = random.Random(0)
    cost = sum(gcost(g) for g in groups)
    best = ([list(g) for g in groups], cost)
    iters = 120000
    for it in range(iters):
        T = 2000.0 * (1.0 / 2000.0) ** (it / iters)
        a, b = rng.randrange(CH), rng.randrange(CH)
        if a == b:
            continue
        i, j = rng.randrange(N_CORES), rng.randrange(N_CORES)
        ca0, cb0 = gcost(groups[a]), gcost(groups[b])
        groups[a][i], groups[b][j] = groups[b][j], groups[a][i]
        ca, cb = gcost(groups[a]), gcost(groups[b])
        d = (ca + cb) - (ca0 + cb0)
        if d <= 0 or rng.random() < math.exp(-d / T):
            cost += d
            if cost < best[1]:
                best = ([list(g) for g in groups], cost)
        else:
            groups[a][i], groups[b][j] = groups[b][j], groups[a][i]
    groups = best[0]
    # segment 0 carries the excitation; the program injects x only at
    # chain-0 slots, so its group must be chain 0
    q0 = next(q for q in range(CH) if 0 in groups[q])
    groups[0], groups[q0] = groups[q0], groups[0]
    assign = np.zeros((N_CORES, CH), np.int64)
    for q in range(CH):
        for s in range(N_CORES):
            assign[s, q] = groups[q][s]
    return assign


def _make_plans(seg_wts_neg, assign):
    """Per-slot (q, r) union plans across cores.

    Returns plans[q][r] = (inseg_blocks tuple, has_C), and
    corr_y[q][r] = bool (host fixup positions)."""
    act = _block_act(seg_wts_neg)
    inseg, pre = _split_act(seg_wts_neg)
    plans = []
    corr_y = []
    for q in range(CH):
        segs = [assign[s, q] for s in range(N_CORES)]
        u = inseg[segs].any(0)   # [CPC, 4]
        up = pre[segs].any(0)    # [CPC]
        uc = act[segs].any(0)    # [CPC, 5]
        pq = []
        cq = []
        for r in range(CPC):
            wb = tuple(g for g in range(4) if u[r, g])
            assert wb or up[r], f"slot ({q},{r}) reads nothing"
            pq.append((wb, bool(up[r])))
            cq.append(bool(uc[r, 4]))
        plans.append(pq)
        corr_y.append(cq)
    return plans, corr_y


def _pack_weights(seg_wts_neg, plans, assign, s):
    """Pack core s's in-segment weight blocks, round-major, partition-major
    fp16 layout [128, TOT, 128]."""
    cols = []
    for r in range(CPC):
        for q in range(CH):
            wb, _ = plans[q][r]
            if not wb:
                continue
            blocks = seg_wts_neg[assign[s, q]].reshape(CPC, 5, CHUNK, CHUNK)
            cols.append(blocks[r, list(wb)])  # [nb, 128, 128]
    flat = np.concatenate(cols, 0)            # [TOT, 128k, 128m]
    return np.ascontiguousarray(
        flat.transpose(1, 0, 2)).astype(np.float16)  # [128, TOT, 128]


def _build_cin(seg_wts_neg, plans, basis_j, chain_nr, assign, s):
    """Host-precomputed initial-window contributions: for rounds reading
    before the segment, C[m, :] = sum_preseg_blocks W_g^T @ ring0_col_g.
    Returns per-chain [128, NC_q, nr_q] fp16 arrays."""
    out = []
    for q in range(CH):
        j = int(assign[s, q])
        nr = chain_nr[q]
        bj = int(basis_j[j])
        # full unit-column windows for the 4 pre-segment window chunks
        r0 = np.zeros((4, CHUNK, nr), np.float32)
        for b in range(bj):
            p = (WIN - bj) + b
            r0[p // CHUNK, p % CHUNK, b] = 1.0
        blocks = seg_wts_neg[j].reshape(CPC, 5, CHUNK, CHUNK)
        cs = []
        for r in range(CPC):
            wb, has_c = plans[q][r]
            if not has_c:
                continue
            C = np.zeros((CHUNK, nr), np.float32)
            for g in range(4 - r):
                # block g of chunk r's window is chunk-0-window block r+g
                C += blocks[r, g].astype(np.float32).T @ r0[r + g]
            cs.append(C)
        out.append(np.stack(cs, 1).astype(np.float16) if cs
                   else np.zeros((CHUNK, 0, nr), np.float16))
    return out


def _plan_key(plans):
    return tuple(tuple(pq) for pq in plans)


# ----------------------------------------------------------------------------
# prog TOPS: per-segment transfer operators
# ----------------------------------------------------------------------------

N_WARM = 28  # PE p-state warm-up matmuls issued during the DMA ramp


def _build_tops_nc(plans, tot_blocks, chain_basis):
    key = ("T4", _plan_key(plans), tot_blocks, tuple(chain_basis), N_WARM)
    if key in _NC_CACHE:
        return _NC_CACHE[key]
    chain_nr = []
    chain_pidx = []
    chain_nc = []
    for q in range(CH):
        nr, pidx = _nr_of(int(chain_basis[q]))
        chain_nr.append(nr)
        chain_pidx.append(pidx)
        chain_nc.append(sum(1 for r in range(CPC) if plans[q][r][1]))

    nc = bacc.Bacc("TRN2", target_bir_lowering=False, debug=False,
                   num_devices=N_CORES, enable_partition_id=False)
    wts = nc.dram_tensor("wts", [CHUNK, tot_blocks, CHUNK], F16,
                         kind="ExternalInput")
    xin = nc.dram_tensor("xin", [CHUNK, CH * CPC], F32, kind="ExternalInput")
    ident = nc.dram_tensor("ident", [CHUNK, CHUNK], F16,
                           kind="ExternalInput")
    cin = [
        nc.dram_tensor(f"cin_{q}", [CHUNK, max(chain_nc[q], 1),
                                    chain_nr[q]], F16, kind="ExternalInput")
        for q in range(CH)
    ]
    hout = [
        nc.dram_tensor(f"hout_{q}", [CHUNK, CPC, chain_nr[q]], F16,
                       kind="ExternalOutput")
        for q in range(CH)
    ]

    with tile.TileContext(nc) as tc:
        with (
            tc.tile_pool(name="state", bufs=1) as state,
            tc.tile_pool(name="wpool", bufs=CPC) as wpool,
            tc.tile_pool(name="psum", bufs=8, space="PSUM") as ppool,
        ):
            ring = [state.tile([CHUNK, RING, chain_nr[q]], F16,
                               name=f"ring{q}")
                    for q in range(CH)]
            xin_sb = state.tile([CHUNK, CH * CPC], F32)
            ident_sb = state.tile([CHUNK, CHUNK], F16)
            cin_sb = [state.tile([CHUNK, max(chain_nc[q], 1), chain_nr[q]],
                                 F16, name=f"cin_sb{q}")
                      for q in range(CH)]

            # PE p-state warm-up: independent dummy matmuls keep the PE
            # array clocked up while the input DMAs land
            warm = state.tile([CHUNK, 512], F16)
            nc.vector.memset(warm[:], 0.0)
            # borrow a slot from the chain-1 accumulator tag (PSUM has
            # exactly 8 banks: 4 chains x 2 bufs)
            wpsum = ppool.tile([CHUNK, 512], F32, tag="acc1", bufs=2)
            for _ in range(N_WARM):
                nc.tensor.matmul(wpsum[:], warm[:, 0:CHUNK], warm[:],
                                 start=True, stop=True)

            # All DMA issues on SP/ACT only (Pool's DIRECT2D path is slow
            # and serialized; DVE can't issue).  Ramp-critical first.
            for q in range(CH):
                eng = nc.sync if q % 2 == 0 else nc.scalar
                if chain_nc[q]:
                    # per-column so round r waits only on its own C column
                    for ci in range(chain_nc[q]):
                        eng.dma_start(cin_sb[q][:, ci, :], cin[q][:, ci, :])
            nc.scalar.dma_start(ident_sb[:], ident[:])
            nc.sync.dma_start(xin_sb[:], xin[:])

            # pre-issue the full weight stream (bufs=CPC keeps every round
            # resident in SBUF, decoupling DMA from the compute rate)
            wtiles = []
            woff = 0
            for r in range(CPC):
                nbr = sum(len(plans[q][r][0]) for q in range(CH))
                wtile = wpool.tile([CHUNK, max(nbr, 1), CHUNK], F16, tag="w")
                if nbr:
                    eng = nc.sync if r % 2 == 0 else nc.scalar
                    eng.dma_start(
                        wtile[:, 0:nbr, :], wts[:, woff: woff + nbr, :])
                woff += nbr
                wtiles.append(wtile)

            cidx = [0] * CH
            for r in range(CPC):
                wtile = wtiles[r]
                soff = 0
                for q in range(CH):
                    wb, has_c = plans[q][r]
                    nr = chain_nr[q]
                    rc = r % RING
                    psum = ppool.tile([CHUNK, nr], F32, tag=f"acc{q}",
                                      bufs=2)
                    nmm = len(wb) + int(has_c)
                    i = 0
                    if has_c:
                        nc.tensor.matmul(
                            psum[:], ident_sb[:],
                            cin_sb[q][:, cidx[q], :],
                            start=True, stop=(nmm == 1),
                        )
                        cidx[q] += 1
                        i = 1
                    for k, g in enumerate(wb):
                        col = (r + 4 + g) % RING
                        nc.tensor.matmul(
                            psum[:],
                            wtile[:, soff + k, :],
                            ring[q][:, col, :],
                            start=(i + k == 0),
                            stop=(i + k == nmm - 1),
                        )
                    # serial ring update (weights pre-negated: col = psum + x)
                    # split across ACT / DVE (Pool cannot access PSUM)
                    c1 = (int(0.55 * nr) // 2) * 2
                    nc.scalar.copy(ring[q][:, rc, 0:c1], psum[:, 0:c1])
                    nc.vector.tensor_copy(ring[q][:, rc, c1:nr],
                                          psum[:, c1:nr])
                    if q == 0 and r < 4:
                        slot = r * CH + q
                        pidx = chain_pidx[0]
                        nc.vector.tensor_add(
                            ring[0][:, rc, pidx: pidx + 1],
                            ring[0][:, rc, pidx: pidx + 1],
                            xin_sb[:, slot: slot + 1],
                        )
                    soff += len(wb)
                    # stream the response-operator columns out as soon as
                    # their contiguous ring-column batch is final; the last
                    # rounds go in pairs to shrink the drain tail
                    if r in (3, 7, 11):
                        base = (r - 3) % RING
                        eng = nc.sync if (r + q) % 2 == 0 else nc.scalar
                        eng.dma_start(
                            hout[q][:, r - 3: r + 1, :],
                            ring[q][:, base: base + 4, :],
                        )
                    elif r in (13, 15):
                        base = (r - 1) % RING
                        eng = nc.sync if (r + q) % 2 == 0 else nc.scalar
                        eng.dma_start(
                            hout[q][:, r - 1: r + 1, :],
                            ring[q][:, base: base + 2, :],
                        )
    nc.compile()
    _NC_CACHE[key] = nc
    return nc


# ----------------------------------------------------------------------------
# host orchestration
# ----------------------------------------------------------------------------

def _run(nc, in_maps, tag):
    trace = bool(int(os.environ.get("DIFFKS_TRACE", "0")))
    kw = {}
    tcs = os.environ.get("DIFFKS_TRACE_CORES", "")
    if trace and tcs:
        kw["trace_cores"] = [int(x) for x in tcs.split(",")]
    res = run_bass_kernel_spmd(
        nc, in_maps, core_ids=list(range(len(in_maps))), trace=trace, **kw
    )
    LAST_RESULTS[tag] = res
    return res.results


def kernel(delay_len_frames, raw_coeff_frames, excitation, n_samples):
    n = int(n_samples)
    assert n == N_SAMPLES, f"kernel hardcoded for {N_SAMPLES}, got {n}"
    LAST_RESULTS.clear()

    vals, z_l, x = _preprocess(delay_len_frames, raw_coeff_frames,
                               excitation, n)
    wts, basis = _build_wts(vals, z_l, n)
    n_chunks = n // CHUNK
    assert n_chunks == SEGS * CPC
    xin_cols = np.ascontiguousarray(x.reshape(n_chunks, CHUNK).T)  # [128, nc]

    # fold corrections, then negate everything (update becomes plain copy)
    seg_wts_neg = [-_fold_corr(wts[j * CPC:(j + 1) * CPC])
                   for j in range(SEGS)]
    basis_j = _seg_basis(seg_wts_neg)
    assign = _assign_segments(seg_wts_neg, basis_j)
    inv = {int(assign[s, q]): (s, q)
           for s in range(N_CORES) for q in range(CH)}
    plans, corr_y = _make_plans(seg_wts_neg, assign)
    tot_blocks = sum(len(plans[q][r][0])
                     for r in range(CPC) for q in range(CH))
    chain_basis = [int(max(basis_j[assign[s, q]] for s in range(N_CORES)))
                   for q in range(CH)]
    chain_nr = []
    chain_pidx = []
    for q in range(CH):
        nr, pidx = _nr_of(chain_basis[q])
        chain_nr.append(nr)
        chain_pidx.append(pidx)

    ncT = _build_tops_nc(plans, tot_blocks, chain_basis)
    packed_wts = [_pack_weights(seg_wts_neg, plans, assign, s)
                  for s in range(N_CORES)]
    ident = np.eye(CHUNK, dtype=np.float16)
    in_maps = []
    for s in range(N_CORES):
        xin = np.zeros((CHUNK, CH * CPC), np.float32)
        for r in range(CPC):
            for q in range(CH):
                gchunk = int(assign[s, q]) * CPC + r
                xin[:, r * CH + q] = xin_cols[:, gchunk]
        im = {"wts": packed_wts[s], "xin": xin, "ident": ident}
        cins = _build_cin(seg_wts_neg, plans, basis_j, chain_nr, assign, s)
        for q in range(CH):
            im[f"cin_{q}"] = (cins[q] if cins[q].shape[1] else
                              np.zeros((CHUNK, 1, chain_nr[q]), np.float16))
        in_maps.append(im)
    outsT = _run(ncT, in_maps, "tops")

    # host combine: apply correction fixups to each segment's transfer
    # operator (the last 4 response columns), then chain them (fp32) to get
    # every segment's true initial window
    Hs = {(s, q): outsT[s][f"hout_{q}"].astype(np.float32)
          for s in range(N_CORES) for q in range(CH)}  # [128, CPC, nr_q]
    wins = [np.zeros(WIN, np.float32)]
    for j in range(SEGS):
        s, q = inv[j]
        T = np.array(Hs[(s, q)][:, CPC - 4: CPC, :])   # [128, 4, nr_q]
        blocks = seg_wts_neg[j].reshape(CPC, 5, CHUNK, CHUNK)
        for k in range(4):
            Lc = blocks[CPC - 4 + k, 4][0:CORR]       # negated lhsT [64, 128]
            if np.any(Lc):
                fix = Lc.T @ T[0:CORR, k, :]          # [128, nr]
                T[CORR:, k, :] += fix[CORR:]
        T = T.transpose(1, 0, 2).reshape(WIN, chain_nr[q])
        bj = int(basis_j[j])
        w_next = T[:, :bj] @ wins[j][WIN - bj:] + T[:, chain_pidx[q]]
        wins.append(w_next.astype(np.float32))

    # host apply: y chunks = H @ [w; 1] per segment, plus within-chunk
    # correction fix-ups
    y = np.zeros(n, np.float32)
    for s in range(N_CORES):
        for q in range(CH):
            j = int(assign[s, q])
            wv = np.zeros(chain_nr[q], np.float32)
            bj = int(basis_j[j])
            wv[:bj] = wins[j][WIN - bj:]
            wv[chain_pidx[q]] = 1.0
            yo = Hs[(s, q)] @ wv                       # [128, CPC]
            blocks = seg_wts_neg[j].reshape(CPC, 5, CHUNK, CHUNK)
            for r in range(CPC):
                col = yo[:, r]
                Lc = blocks[r, 4][0:CORR]           # negated lhsT [64, 128]
                if np.any(Lc):
                    fix = Lc.T @ col[0:CORR]
                    col = col.copy()
                    col[CORR:] += fix[CORR:]
                gchunk = j * CPC + r
                y[gchunk * CHUNK:(gchunk + 1) * CHUNK] = col
    return y.astype(np.float32)
